# revision 35
# baseline (speedup 1.0000x reference)
"""AutoCorrelation (Autoformer-style) Trainium2 Bass kernel, v2.

Sharding: data-parallel over batch — 8 batch elements -> 8 NeuronCores, no
collectives. Each core computes its [2048, 128] output slice independently.

Algorithm (same math as v1, validated vs the reference):
  * Folded real-DFT matrix pair C,S = cos/sin(2*pi*i*j/2048) [1024x1024] in
    fp16 (halves HBM traffic; matmuls run 1 cycle/row with fp32 PSUM).
  * x is folded host-side (xc = x[0:1024]+xr, xs = xr-x, DC row halved);
    Q/K/V obtained by projecting the hidden spectrum; biases enter the DC
    bin; Nyquist bin carried separately.
  * wq/wk are host-scaled by 1/32 so Q*K products fit fp16; the exact 2^-2
    restore lands only on the top-k logits (ordering is scale-invariant).
  * Q/K spectra are evacuated PSUM->SBUF fp16 (split across Scalar/Vector/
    GpSimd) and the complex products + head accumulation run as
    scalar_tensor_tensor ops at 4x DVE rate (fp16, packed, SBUF).
  * top-22 per channel: 3 rounds of max8 + match_replace on fp32; delay
    remapped arithmetically from the permuted acm layout (E-O | E+O).
  * circular roll of V = per-channel phase multiply in frequency domain;
    softmax weight and 2/T fold into the fp16 phase tables.
  * output projection runs with wo stationary (16 LDWEIGHTS, long moving
    streams), then 16 PE transposes restore the [f, e] layout for the
    final inverse DFT.
"""
import os
import sys
import types
from contextlib import ExitStack

sys.path.insert(0, "/opt/trn_rl_repo")

import numpy as np

import concourse.bass as bass
import concourse.mybir as mybir
from concourse import bacc
from concourse.tile import TileContext
from concourse.bass_utils import run_bass_kernel_spmd

B, T, E, H = 8, 2048, 128, 4
NF = 1024
NCH = 8
AL = mybir.AluOpType
DT = mybir.dt
AF = mybir.ActivationFunctionType
AX = mybir.AxisListType

_CACHE = {}
LAST_EXEC_NS = None


def _wire_ntff_hook():
    if "antenv.axon_hooks" in sys.modules:
        return
    try:
        mod = types.ModuleType("antenv.axon_hooks")
        _h = [None]
        mod.set_axon_ntff_profile_hook = lambda h: _h.__setitem__(0, h)
        mod.get_axon_ntff_profile_hook = lambda: _h[0]
        sys.modules["antenv.axon_hooks"] = mod
        import antenv
        antenv.axon_hooks = mod
        from trn_agent_boot.trn_boot import _ntff_profile_via_ctypes
        mod.set_axon_ntff_profile_hook(_ntff_profile_via_ctypes("/opt/axon/libaxon_pjrt.so"))
    except Exception:
        pass


def _host_consts():
    i = np.arange(NF, dtype=np.float64)
    ang = np.outer(i, i) * (2.0 * np.pi / T)

    def chunk(a):  # [1024,1024] -> [128, 8*1024] chunk-major
        return np.ascontiguousarray(
            a.reshape(NCH, 128, NF).transpose(1, 0, 2).reshape(128, NCH * NF))

    return {
        "cs": chunk(np.cos(ang).astype(np.float16)),
        "sn": chunk(np.sin(ang).astype(np.float16)),
        "altf": ((-1.0) ** np.arange(NF)).astype(np.float16)[None, :],
        "altp": ((-1.0) ** np.arange(128)).astype(np.float16)[:, None],
        "one": np.ones((1, 1), np.float16),
        "mhalf": np.full((1, 1), -0.5, np.float16),
        "mhrow": np.full((1, NF), -0.5, np.float16),
        "ident": np.eye(128, dtype=np.float32),
    }


def _build():
    nc = bacc.Bacc("TRN2", target_bir_lowering=False, debug=False, num_devices=1)
    f32, f32r, f16, i32, u32 = DT.float32, DT.float32r, DT.float16, DT.int32, DT.uint32

    # all 2D tensors ship host-pre-chunked: [partition, chunk-major free]
    xc_d = nc.dram_tensor("xc", [128, NCH * E], f16, kind="ExternalInput")
    xs_d = nc.dram_tensor("xs", [128, NCH * E], f16, kind="ExternalInput")
    xnyq_d = nc.dram_tensor("xnyq", [1, E], f16, kind="ExternalInput")
    cs_d = nc.dram_tensor("cs", [128, NCH * NF], f16, kind="ExternalInput")
    sn_d = nc.dram_tensor("sn", [128, NCH * NF], f16, kind="ExternalInput")
    altf_d = nc.dram_tensor("altf", [1, NF], f16, kind="ExternalInput")
    altp_d = nc.dram_tensor("altp", [128, 1], f16, kind="ExternalInput")
    one_d = nc.dram_tensor("one", [1, 1], f16, kind="ExternalInput")
    mhalf_d = nc.dram_tensor("mhalf", [1, 1], f16, kind="ExternalInput")
    mhrow_d = nc.dram_tensor("mhrow", [1, NF], f16, kind="ExternalInput")
    id_d = nc.dram_tensor("ident", [128, 128], f32r, kind="ExternalInput")
    wqk_d = nc.dram_tensor("wqk", [128, H * 256], f16, kind="ExternalInput")  # pre-scaled 1/32
    wv_d = nc.dram_tensor("wv", [128, H * E], f16, kind="ExternalInput")
    wo_d = nc.dram_tensor("wo", [128, H * E], f16, kind="ExternalInput")
    bqk_d = nc.dram_tensor("bqk", [1, 2 * H * E], f32, kind="ExternalInput")  # (T/32)*bq | (T/32)*bk
    bv_d = nc.dram_tensor("bv", [E, H], f32, kind="ExternalInput")        # T*bv
    bo_d = nc.dram_tensor("bo", [E, 1], f32, kind="ExternalInput")
    lo_d = nc.dram_tensor("out_lo", [E, NF], f32, kind="ExternalOutput")
    hi_d = nc.dram_tensor("out_hi", [E, NF], f32, kind="ExternalOutput")
    o1024_d = nc.dram_tensor("out_1024", [E, 1], f32, kind="ExternalOutput")

    with TileContext(nc) as tc, ExitStack() as ctx:
        pool = ctx.enter_context(tc.tile_pool(name="main", bufs=1))
        pool2 = ctx.enter_context(tc.tile_pool(name="rot", bufs=2))
        pqk = ctx.enter_context(tc.tile_pool(name="pqk", bufs=2, space="PSUM"))    # [128,1024] tiles
        pb = ctx.enter_context(tc.tile_pool(name="pb", bufs=3, space="PSUM"))      # [128,512] tiles
        prow = ctx.enter_context(tc.tile_pool(name="psrow", bufs=1, space="PSUM"))

        # ---------------- loads ----------------
        xc = pool.tile([128, NCH * E], f16, tag="xc")
        xs = pool.tile([128, NCH * E], f16, tag="xs")
        nc.sync.dma_start(xc[:], xc_d[:])
        nc.sync.dma_start(xs[:], xs_d[:])
        xnyq = pool.tile([1, E], f16, tag="xnyq")
        nc.sync.dma_start(xnyq[:], xnyq_d[:])
        wqk_sb = pool.tile([128, H * 256], f16, tag="wqk")
        nc.sync.dma_start(wqk_sb[:], wqk_d[:])
        altf_sb = pool.tile([1, NF], f16, tag="altf")
        nc.sync.dma_start(altf_sb[:], altf_d[:])
        altp_sb = pool.tile([128, 1], f16, tag="altp")
        nc.sync.dma_start(altp_sb[:], altp_d[:])
        one_sb = pool.tile([1, 1], f16, tag="one")
        nc.sync.dma_start(one_sb[:], one_d[:])

        # big DFT matrices: column-half DMAs so each DFT half starts sooner
        cs_sb = pool.tile([128, NCH * NF], f16, tag="cs")
        sn_sb = pool.tile([128, NCH * NF], f16, tag="sn")
        for half in range(2):
            for t_sb, t_d in ((cs_sb, cs_d), (sn_sb, sn_d)):
                dst = t_sb[:].rearrange("p (a f) -> p a f", a=NCH)[:, :, half * 512:(half + 1) * 512]
                src = t_d[:].rearrange("p (a f) -> p a f", a=NCH)[:, :, half * 512:(half + 1) * 512]
                nc.sync.dma_start(dst, src)

        wv_sb = pool.tile([128, H * E], f16, tag="wv")
        nc.sync.dma_start(wv_sb[:], wv_d[:])
        wo_sb = pool.tile([128, H * E], f16, tag="wo")
        nc.sync.dma_start(wo_sb[:], wo_d[:])
        mhalf_sb = pool.tile([1, 1], f16, tag="mhalf")
        nc.sync.dma_start(mhalf_sb[:], mhalf_d[:])
        mhrow_sb = pool.tile([1, NF], f16, tag="mhrow")
        nc.sync.dma_start(mhrow_sb[:], mhrow_d[:])
        id_sb = pool.tile([128, 128], f32r, tag="ident")
        nc.sync.dma_start(id_sb[:], id_d[:])
        bqk_sb = pool.tile([1, 2 * H * E], f32, tag="bqk")
        nc.sync.dma_start(bqk_sb[:], bqk_d[:])
        bv_sb = pool.tile([128, H], f32, tag="bv")
        nc.sync.dma_start(bv_sb[:], bv_d[:])
        bo_sb = pool.tile([128, 1], f32, tag="bo")
        nc.sync.dma_start(bo_sb[:], bo_d[:])

        # iota/phase prep early (gpsimd idle at start)
        negpi = pool.tile([128, 1], f32, tag="negpi")
        nc.gpsimd.memset(negpi[:], float(-np.pi))
        io_i = pool.tile([128, NF], i32, tag="ioi")
        nc.gpsimd.iota(io_i[:], pattern=[[1, NF]], base=0, channel_multiplier=0)
        io_f = pool.tile([128, NF], f32, tag="iof")
        nc.vector.tensor_copy(io_f[:], io_i[:])

        # ---------------- forward DFT ----------------
        hre = pool.tile([128, NF], f16, tag="hre")
        him = pool.tile([128, NF], f16, tag="him")
        hn = pool.tile([128, 1], f16, tag="hn")
        for s in range(2):
            sl = slice(s * 512, (s + 1) * 512)
            hre_ps = pb.tile([128, 512], f32, tag="b512")
            for a in range(NCH):
                nc.tensor.matmul(hre_ps[:], xc[:, a * E:(a + 1) * E],
                                 cs_sb[:, a * NF + s * 512: a * NF + (s + 1) * 512],
                                 start=(a == 0), stop=False)
            nc.tensor.matmul(hre_ps[:], xnyq[:], altf_sb[:, sl], start=False, stop=True)
            if s == 0:
                nc.scalar.copy(hre[:, sl], hre_ps[:])
            else:
                nc.vector.tensor_copy(hre[:, sl], hre_ps[:])
        for s in range(2):
            sl = slice(s * 512, (s + 1) * 512)
            him_ps = pb.tile([128, 512], f32, tag="b512")
            for a in range(NCH):
                nc.tensor.matmul(him_ps[:], xs[:, a * E:(a + 1) * E],
                                 sn_sb[:, a * NF + s * 512: a * NF + (s + 1) * 512],
                                 start=(a == 0), stop=(a == NCH - 1))
            if s == 0:
                nc.scalar.copy(him[:, sl], him_ps[:])
            else:
                nc.vector.tensor_copy(him[:, sl], him_ps[:])
        hn_ps = prow.tile([128, 1], f32, tag="row")
        for a in range(NCH):
            nc.tensor.matmul(hn_ps[:], xc[:, a * E:(a + 1) * E], altp_sb[:],
                             start=(a == 0), stop=False)
        nc.tensor.matmul(hn_ps[:], xnyq[:], one_sb[:], start=False, stop=True)
        nc.vector.tensor_copy(hn[:], hn_ps[:])

        # ---------------- QK projections + products (fp16, 4x stt) ----------------
        # per (h, comp): two psum tiles of [128,1024] each holding 4 chunks of
        # (q 128 | k 128); evacuated to packed fp16 qr/kr/qi/ki tiles.
        pre_t = pool.tile([128, NCH * E], f16, tag="pre")    # heads 0..2
        pim_t = pool.tile([128, NCH * E], f16, tag="pim")
        pre3 = pool.tile([128, NCH * E], f16, tag="pre3")    # head 3 alone
        pim3 = pool.tile([128, NCH * E], f16, tag="pim3")
        qn_row = pool.tile([1, E], f32, tag="qnrow")
        kn_row = pool.tile([1, E], f32, tag="knrow")
        pn_row = pool.tile([1, E], f32, tag="pnrow")
        pn_f16 = pool.tile([1, E], f16, tag="pnf16")
        pnw = pool.tile([1, E], f32, tag="pnw")
        vn_cols = pool.tile([128, H], f32, tag="vncols")
        vre_t, vim_t = {}, {}

        evac_engs = [nc.scalar, nc.scalar, nc.vector, nc.scalar]
        for h in range(H):
            if True:
                qk = {}
                for c in range(2):
                    hsrc = hre if c == 0 else him
                    for g in range(2):
                        ps = pqk.tile([128, 1024], f32, tag="qk1024")
                        for jj in range(4):
                            j = g * 4 + jj
                            nc.tensor.matmul(ps[:, jj * 256:(jj + 1) * 256],
                                             hsrc[:, j * 128:(j + 1) * 128],
                                             wqk_sb[:, h * 256:(h + 1) * 256],
                                             start=True, stop=True)
                        if c == 0 and g == 0:
                            nc.vector.tensor_add(ps[0:1, 0:128], ps[0:1, 0:128],
                                                 bqk_sb[0:1, h * E:(h + 1) * E])
                            nc.vector.tensor_add(ps[0:1, 128:256], ps[0:1, 128:256],
                                                 bqk_sb[0:1, H * E + h * E:H * E + (h + 1) * E])
                        v3 = ps[:].rearrange("p (jj k e) -> k p jj e", jj=4, k=2)
                        qt = pool2.tile([128, NF], f16, tag=f"q{c}")
                        kt = pool2.tile([128, NF], f16, tag=f"k{c}")
                        if g == 0:
                            qk[c] = (qt, kt)
                        else:
                            qt, kt = qk[c]
                        r2 = lambda ap: ap.rearrange("p (jj e) -> p jj e", jj=4)
                        e0 = evac_engs[(h + 2 * c + g) % 4]
                        e1 = evac_engs[(h + 2 * c + g + 2) % 4]
                        if hasattr(e0, "tensor_copy"):
                            e0.tensor_copy(r2(qt[:, g * 512:(g + 1) * 512]), v3[0])
                        else:
                            e0.copy(r2(qt[:, g * 512:(g + 1) * 512]), v3[0])
                        if hasattr(e1, "tensor_copy"):
                            e1.tensor_copy(r2(kt[:, g * 512:(g + 1) * 512]), v3[1])
                        else:
                            e1.copy(r2(kt[:, g * 512:(g + 1) * 512]), v3[1])
                (qr, kr), (qi, ki) = qk[0], qk[1]
                qr, kr, qi, ki = qr[:], kr[:], qi[:], ki[:]
                t1 = pool2.tile([128, NF], f16, tag="t1")
                t2 = pool2.tile([128, NF], f16, tag="t2")
                t3 = pool2.tile([128, NF], f16, tag="t3")
                t4 = pool2.tile([128, NF], f16, tag="t4")
                nc.vector.tensor_tensor(t1[:], qr, kr, AL.mult)
                nc.vector.tensor_tensor(t2[:], qi, ki, AL.mult)
                nc.vector.tensor_tensor(t3[:], qi, kr, AL.mult)
                nc.vector.tensor_tensor(t4[:], qr, ki, AL.mult)
                if h == 0:
                    nc.vector.tensor_add(pre_t[:], t1[:], t2[:])
                    nc.vector.tensor_sub(pim_t[:], t3[:], t4[:])
                elif h < 3:
                    nc.vector.tensor_add(pre_t[:], pre_t[:], t1[:])
                    nc.vector.tensor_add(pre_t[:], pre_t[:], t2[:])
                    nc.vector.tensor_add(pim_t[:], pim_t[:], t3[:])
                    nc.vector.tensor_sub(pim_t[:], pim_t[:], t4[:])
                else:
                    nc.vector.tensor_add(pre3[:], t1[:], t2[:])
                    nc.vector.tensor_sub(pim3[:], t3[:], t4[:])

                # V spectra for this head (fills PE while vector runs products)
                for c in range(2):
                    hsrc = hre if c == 0 else him
                    vt = pool2.tile([128, NF], f16, tag=f"v{c}{h % 2}")
                    (vre_t if c == 0 else vim_t)[h] = vt
                    for sv in range(2):
                        v_ps = pb.tile([128, 512], f32, tag="b512")
                        nc.tensor.matmul(v_ps[:], wv_sb[:, h * E:(h + 1) * E],
                                         hsrc[:, sv * 512:(sv + 1) * 512], start=True, stop=True)
                        if c == 0 and sv == 0:
                            nc.vector.tensor_add(v_ps[:, 0:1], v_ps[:, 0:1], bv_sb[:, h:h + 1])
                        nc.scalar.copy(vt[:, sv * 512:(sv + 1) * 512], v_ps[:])
                vn_ps = prow.tile([128, 1], f32, tag="row")
                nc.tensor.matmul(vn_ps[:], wv_sb[:, h * E:(h + 1) * E], hn[:], start=True, stop=True)
                nc.scalar.copy(vn_cols[:, h:h + 1], vn_ps[:])

                # Nyquist rows: qn = hn^T @ wq_h ; kn = hn^T @ wk_h (scaled /32 each)
                r_ps = prow.tile([1, 256], f32, tag="row")
                nc.tensor.matmul(r_ps[:], hn[:], wqk_sb[:, h * 256:(h + 1) * 256],
                                 start=True, stop=True)
                nc.scalar.copy(qn_row[:], r_ps[:, 0:128])
                nc.scalar.copy(kn_row[:], r_ps[:, 128:256])
                # unscaled-eo units: pn += 0.5 * qn' * kn'
                if h == 0:
                    nc.vector.scalar_tensor_tensor(pn_row[:], qn_row[:], 0.5, kn_row[:], AL.mult, AL.mult)
                else:
                    nc.vector.scalar_tensor_tensor(pnw[:], qn_row[:], 0.5, kn_row[:], AL.mult, AL.mult)
                    nc.vector.tensor_add(pn_row[:], pn_row[:], pnw[:])
        nc.vector.tensor_copy(pn_f16[:], pn_row[:])

        # ---------------- acm inverse (unscaled by 4x; fixed at softmax) ----------------
        eo_sb = pool.tile([128, T], f16, tag="eo")
        for s in range(2):
            sl = slice(s * 512, (s + 1) * 512)
            e_ps = pb.tile([128, 512], f32, tag="b512")
            o_ps = pb.tile([128, 512], f32, tag="b512")
            # heads 0-2 chunks issue first (run while h3 products finish)
            for j in range(NCH):
                nc.tensor.matmul(e_ps[:], pre_t[:, j * E:(j + 1) * E],
                                 cs_sb[:, j * NF + s * 512: j * NF + (s + 1) * 512],
                                 start=(j == 0), stop=False)
            for j in range(NCH):
                nc.tensor.matmul(o_ps[:], pim_t[:, j * E:(j + 1) * E],
                                 sn_sb[:, j * NF + s * 512: j * NF + (s + 1) * 512],
                                 start=(j == 0), stop=False)
            for j in range(NCH):
                nc.tensor.matmul(e_ps[:], pre3[:, j * E:(j + 1) * E],
                                 cs_sb[:, j * NF + s * 512: j * NF + (s + 1) * 512],
                                 start=False, stop=False)
            nc.tensor.matmul(e_ps[:], pn_f16[:], altf_sb[:, sl], start=False, stop=True)
            for j in range(NCH):
                nc.tensor.matmul(o_ps[:], pim3[:, j * E:(j + 1) * E],
                                 sn_sb[:, j * NF + s * 512: j * NF + (s + 1) * 512],
                                 start=False, stop=(j == NCH - 1))
            e_sb = pool2.tile([128, 512], f32, tag="ecp")
            nc.scalar.copy(e_sb[:], e_ps[:])
            nc.vector.tensor_sub(eo_sb[:, sl], e_sb[:], o_ps[:])
            if s == 0:
                nc.vector.tensor_add(eo_sb[:, NF + 1:NF + 512], e_sb[:, 1:], o_ps[:, 1:])
            else:
                nc.vector.tensor_add(eo_sb[:, NF + 512:2 * NF], e_sb[:], o_ps[:])
        a1024_ps = prow.tile([128, 1], f32, tag="row")
        for j in range(NCH):
            nc.tensor.matmul(a1024_ps[:], pre_t[:, j * E:(j + 1) * E], altp_sb[:],
                             start=(j == 0), stop=False)
        for j in range(NCH):
            nc.tensor.matmul(a1024_ps[:], pre3[:, j * E:(j + 1) * E], altp_sb[:],
                             start=False, stop=False)
        nc.tensor.matmul(a1024_ps[:], pn_f16[:], one_sb[:], start=False, stop=True)
        nc.scalar.copy(eo_sb[:, NF:NF + 1], a1024_ps[:])

        # ---------------- top-k ----------------
        vals = pool.tile([128, 24], f16, tag="vals")
        nc.vector.max(vals[:, 0:8], eo_sb[:])
        nc.vector.match_replace(eo_sb[:], vals[:, 0:8], eo_sb[:], -60000.0)
        nc.vector.max(vals[:, 8:16], eo_sb[:])
        nc.vector.match_replace(eo_sb[:], vals[:, 8:16], eo_sb[:], -60000.0)
        nc.vector.max(vals[:, 16:24], eo_sb[:])
        idx8 = pool.tile([128, 8], u32, tag="idx8")
        nc.vector.max_index(idx8[:], vals[:, 16:24], eo_sb[:])

        c_i = pool.tile([128, 1], i32, tag="ci")
        nc.vector.tensor_copy(c_i[:], idx8[:, 5:6].bitcast(i32))
        c_neg = pool.tile([128, 1], i32, tag="cneg")
        nc.vector.tensor_scalar(c_neg[:], c_i[:], -1, 3072, AL.mult, AL.add)
        mask = pool.tile([128, 1], i32, tag="mask")
        nc.vector.tensor_scalar(mask[:], c_i[:], 1024, None, AL.is_gt)
        d_i = pool.tile([128, 1], i32, tag="di")
        nc.vector.select(d_i[:], mask[:], c_neg[:], c_i[:])
        d_f = pool.tile([128, 1], f32, tag="df")
        nc.vector.tensor_copy(d_f[:], d_i[:])

        # softmax over 0.25-restored logits
        negv0 = pool.tile([128, 1], f32, tag="negv0")
        nc.vector.tensor_scalar_mul(negv0[:], vals[:, 0:1], -0.25)
        expv = pool.tile([128, 24], f32, tag="expv")
        nc.scalar.activation(expv[:, 0:22], vals[:, 0:22], AF.Exp, bias=negv0[:], scale=0.25)
        den = pool.tile([128, 1], f32, tag="den")
        nc.vector.tensor_reduce(den[:], expv[:, 0:22], AX.X, AL.add)
        rden = pool.tile([128, 1], f32, tag="rden")
        nc.vector.reciprocal(rden[:], den[:])
        wgt = pool.tile([128, 1], f32, tag="wgt")
        nc.vector.tensor_mul(wgt[:], expv[:, 21:22], rden[:])

        # ---------------- phases ----------------
        cw = pool.tile([128, NF], f16, tag="cw")
        sw = pool.tile([128, NF], f16, tag="sw")
        for off, dst in ((0.0, sw), (512.0, cw)):
            mf = pool2.tile([128, NF], f32, tag="ptmp")
            if off == 0.0:
                nc.vector.tensor_scalar(mf[:], io_f[:], d_f[:], None, AL.mult)
            else:
                nc.vector.tensor_scalar(mf[:], io_f[:], d_f[:], off, AL.mult, AL.add)
            mi = pool2.tile([128, NF], i32, tag="ptmp")
            nc.vector.tensor_copy(mi[:], mf[:])
            nc.vector.tensor_scalar(mi[:], mi[:], 2047, None, AL.bitwise_and)
            mf2 = pool2.tile([128, NF], f32, tag="ptmp")
            nc.vector.tensor_copy(mf2[:], mi[:])
            ph = pool2.tile([128, NF], f32, tag="ptmp")
            nc.scalar.activation(ph[:], mf2[:], AF.Sin,
                                 scale=float(np.pi / 1024.0), bias=negpi[:])
            nc.vector.tensor_scalar(dst[:], ph[:], wgt[:], -2.0 / T, AL.mult, AL.mult)
        swn = pool.tile([128, NF], f16, tag="swn")
        nc.vector.tensor_scalar_mul(swn[:], sw[:], -1.0)
        # nyquist scale: (1-2*(d&1)) * wgt / T
        par_i = pool.tile([128, 1], i32, tag="par")
        nc.vector.tensor_scalar(par_i[:], d_i[:], 1, None, AL.bitwise_and)
        parf = pool.tile([128, 1], f32, tag="parf")
        nc.vector.tensor_copy(parf[:], par_i[:])
        nc.vector.tensor_scalar(parf[:], parf[:], -2.0, 1.0, AL.mult, AL.add)
        nys = pool.tile([128, 1], f32, tag="nys")
        nc.vector.tensor_scalar(nys[:], parf[:], wgt[:], 1.0 / T, AL.mult, AL.mult)

        # ---------------- phase multiply + output projection (wo stationary) ----------------
        gn_cols = pool.tile([128, H], f16, tag="gncols")
        og_re = pqk.tile([128, 1024], f32, tag="qk1024")
        og_im = pqk.tile([128, 1024], f32, tag="qk1024")
        for h in range(H):
            vre, vim = vre_t[h], vim_t[h]
            nc.vector.tensor_scalar(gn_cols[:, h:h + 1], vn_cols[:, h:h + 1], nys[:], None, AL.mult)
            m1 = pool2.tile([128, NF], f16, tag="m1")
            m2 = pool2.tile([128, NF], f16, tag="m2")
            m3 = pool2.tile([128, NF], f16, tag="m3")
            m4 = pool2.tile([128, NF], f16, tag="m4")
            nc.vector.tensor_tensor(m1[:], vre[:], cw[:], AL.mult)
            nc.vector.tensor_tensor(m2[:], vim[:], swn[:], AL.mult)
            nc.vector.tensor_tensor(m3[:], vre[:], sw[:], AL.mult)
            nc.vector.tensor_tensor(m4[:], vim[:], cw[:], AL.mult)
            for s in range(2):
                sl = slice(s * 512, (s + 1) * 512)
                nc.tensor.matmul(og_re[:, sl], wo_sb[:, h * E:(h + 1) * E], m1[:, sl],
                                 start=(h == 0), stop=False)
                nc.tensor.matmul(og_re[:, sl], wo_sb[:, h * E:(h + 1) * E], m2[:, sl],
                                 start=False, stop=(h == H - 1))
                nc.tensor.matmul(og_im[:, sl], wo_sb[:, h * E:(h + 1) * E], m3[:, sl],
                                 start=(h == 0), stop=False)
                nc.tensor.matmul(og_im[:, sl], wo_sb[:, h * E:(h + 1) * E], m4[:, sl],
                                 start=False, stop=(h == H - 1))
        ofn_ps = prow.tile([1, E], f32, tag="row")
        for h in range(H):
            nc.tensor.matmul(ofn_ps[:], gn_cols[:, h:h + 1], wo_sb[:, h * E:(h + 1) * E],
                             start=(h == 0), stop=(h == H - 1))
        ofn_row = pool.tile([1, E], f16, tag="ofnrow")
        nc.vector.tensor_copy(ofn_row[:], ofn_ps[:])

        # transpose og [e'', f] -> of [f, e''] via PE (f32r) interleaved with
        # the final-inverse accumulation (e2/o2 span both s-halves in psum)
        g2_re = pool.tile([128, 1024], f32r, tag="g2re")
        g2_im = pool.tile([128, 1024], f32r, tag="g2im")
        nc.scalar.copy(g2_re[:], og_re[:])
        nc.scalar.copy(g2_im[:], og_im[:])
        of_re = pool.tile([128, NCH * E], f16, tag="ofre")
        of_im = pool.tile([128, NCH * E], f16, tag="ofim")
        e2_ps = pqk.tile([128, 1024], f32, tag="qk1024")
        o2_ps = pqk.tile([128, 1024], f32, tag="qk1024")
        id_r = id_sb[:]
        for half in range(4):
            tp = pb.tile([128, 512], f32, tag="b512")
            src = g2_re if half < 2 else g2_im
            dst = of_re if half < 2 else of_im
            base = (half % 2) * 512
            for q in range(4):
                j = (half % 2) * 4 + q
                nc.tensor.transpose(tp[:, q * 128:(q + 1) * 128].bitcast(f32r),
                                    src[:, j * 128:(j + 1) * 128], id_r)
            if half % 2 == 0:
                nc.scalar.copy(dst[:, base:base + 512], tp[:])
            else:
                nc.vector.tensor_copy(dst[:, base:base + 512], tp[:])
            ps_t = e2_ps if half < 2 else o2_ps
            tbl = cs_sb if half < 2 else sn_sb
            for q in range(4):
                j = (half % 2) * 4 + q
                for s in range(2):
                    nc.tensor.matmul(ps_t[:, s * 512:(s + 1) * 512],
                                     dst[:, j * E:(j + 1) * E],
                                     tbl[:, j * NF + s * 512: j * NF + (s + 1) * 512],
                                     start=(half % 2 == 0 and q == 0),
                                     stop=(half == 3 and q == 3))
            if half == 1:
                for s in range(2):
                    sl = slice(s * 512, (s + 1) * 512)
                    nc.tensor.matmul(e2_ps[:, sl], ofn_row[:], altf_sb[:, sl],
                                     start=False, stop=False)
                    nc.tensor.matmul(e2_ps[:, sl], of_re[0:1, 0:E], mhrow_sb[:, sl],
                                     start=False, stop=True)

        for s in range(2):
            sl = slice(s * 512, (s + 1) * 512)
            ep_sb = pool2.tile([128, 512], f32, tag="ecp")
            nc.scalar.copy(ep_sb[:], e2_ps[:, sl])
            out_lo = pool2.tile([128, 512], f32, tag="outlo")
            out_hi = pool2.tile([128, 512], f32, tag="outlo")
            nc.vector.scalar_tensor_tensor(out_lo[:], ep_sb[:], bo_sb[:], o2_ps[:, sl], AL.add, AL.subtract)
            nc.vector.scalar_tensor_tensor(out_hi[:], ep_sb[:], bo_sb[:], o2_ps[:, sl], AL.add, AL.add)
            nc.sync.dma_start(lo_d[:, sl], out_lo[:])
            nc.sync.dma_start(hi_d[:, sl], out_hi[:])
        # t = 1024 row
        o1_ps = prow.tile([128, 1], f32, tag="row")
        for j in range(NCH):
            nc.tensor.matmul(o1_ps[:], of_re[:, j * E:(j + 1) * E], altp_sb[:],
                             start=(j == 0), stop=False)
        nc.tensor.matmul(o1_ps[:], ofn_row[:], one_sb[:], start=False, stop=False)
        nc.tensor.matmul(o1_ps[:], of_re[0:1, 0:E], mhalf_sb[:], start=False, stop=True)
        o1_sb = pool.tile([128, 1], f32, tag="o1sb")
        nc.vector.tensor_scalar(o1_sb[:], o1_ps[:], bo_sb[:], None, AL.add)
        nc.sync.dma_start(o1024_d[:], o1_sb[:])

    nc.compile()
    return nc


def _get_nc():
    if "nc" not in _CACHE:
        _wire_ntff_hook()
        _CACHE["nc"] = _build()
    return _CACHE["nc"]


def kernel(hidden_states, wq, bq, wk, bk, wv, bv, wo, bo):
    global LAST_EXEC_NS
    nc = _get_nc()
    consts = _CACHE.setdefault("consts", _host_consts())

    def chunked(a):
        # [1024, W] -> [128, 8*W] with chunk-major columns (device layout)
        W = a.shape[1]
        return np.ascontiguousarray(
            a.reshape(NCH, 128, W).transpose(1, 0, 2).reshape(128, NCH * W))

    hs = np.ascontiguousarray(hidden_states, dtype=np.float32)
    wqk = np.ascontiguousarray(
        (np.concatenate([wq.transpose(2, 0, 1), wk.transpose(2, 0, 1)], axis=2)
         * (1.0 / 32.0)).transpose(1, 0, 2).reshape(128, H * 256)).astype(np.float16)
    wv_h = np.ascontiguousarray(
        wv.transpose(2, 0, 1).transpose(1, 0, 2).reshape(128, H * E)).astype(np.float16)
    wo_h = np.ascontiguousarray(
        wo.transpose(1, 0, 2).transpose(1, 0, 2).reshape(128, H * E)).astype(np.float16)
    bqk = (np.concatenate([(T * bq.T).reshape(-1), (T * bk.T).reshape(-1)])[None, :]
           * (1.0 / 32.0)).astype(np.float32)                                  # [1, 2*H*E]
    bv_s = np.ascontiguousarray(T * bv, dtype=np.float32)                      # [E, H]
    bo_c = np.ascontiguousarray(bo, dtype=np.float32)[:, None]                 # [E, 1]

    in_maps = []
    for b in range(B):
        x = hs[b]
        xr = np.concatenate([x[0:1], x[:0:-1]])[:NF]
        xc = (x[:NF] + xr)
        xc[0] *= 0.5
        xs = (xr - x[:NF])
        in_maps.append({
            "xc": chunked(xc).astype(np.float16), "xs": chunked(xs).astype(np.float16),
            "xnyq": x[NF:NF + 1].astype(np.float16),
            "cs": consts["cs"], "sn": consts["sn"], "altf": consts["altf"],
            "altp": consts["altp"], "one": consts["one"], "mhalf": consts["mhalf"],
            "mhrow": consts["mhrow"], "ident": consts["ident"],
            "wqk": wqk, "wv": wv_h, "wo": wo_h, "bqk": bqk, "bv": bv_s, "bo": bo_c,
        })

    trace = bool(int(os.environ.get("BASS_KERNEL_TRACE", "0")))
    res = run_bass_kernel_spmd(nc, in_maps, core_ids=list(range(B)), trace=trace)
    LAST_EXEC_NS = res.exec_time_ns
    _CACHE["last_res"] = res

    out = np.empty((B, T, E), dtype=np.float32)
    for b in range(B):
        r = res.results[b]
        out[b, 0:NF] = r["out_lo"].T
        out[b, NF] = r["out_1024"][:, 0]
        out[b, NF + 1:] = r["out_hi"][:, 1:NF][:, ::-1].T
    return out


# revision 36
# speedup vs baseline: 1.1148x; 1.1148x over previous
"""AutoCorrelation (Autoformer-style) Trainium2 Bass kernel, v2.

Sharding: data-parallel over batch — 8 batch elements -> 8 NeuronCores, no
collectives. Each core computes its [2048, 128] output slice independently.

Algorithm (same math as v1, validated vs the reference):
  * Folded real-DFT matrix pair C,S = cos/sin(2*pi*i*j/2048) [1024x1024] in
    fp16 (halves HBM traffic; matmuls run 1 cycle/row with fp32 PSUM).
  * x is folded host-side (xc = x[0:1024]+xr, xs = xr-x, DC row halved);
    Q/K/V obtained by projecting the hidden spectrum; biases enter the DC
    bin; Nyquist bin carried separately.
  * wq/wk are host-scaled by 1/32 so Q*K products fit fp16; the exact 2^-2
    restore lands only on the top-k logits (ordering is scale-invariant).
  * Q/K spectra are evacuated PSUM->SBUF fp16 (split across Scalar/Vector/
    GpSimd) and the complex products + head accumulation run as
    scalar_tensor_tensor ops at 4x DVE rate (fp16, packed, SBUF).
  * top-22 per channel: 3 rounds of max8 + match_replace on fp32; delay
    remapped arithmetically from the permuted acm layout (E-O | E+O).
  * circular roll of V = per-channel phase multiply in frequency domain;
    softmax weight and 2/T fold into the fp16 phase tables.
  * output projection runs with wo stationary (16 LDWEIGHTS, long moving
    streams), then 16 PE transposes restore the [f, e] layout for the
    final inverse DFT.
"""
import os
import sys
import types
from contextlib import ExitStack

sys.path.insert(0, "/opt/trn_rl_repo")

import numpy as np

import concourse.bass as bass
import concourse.mybir as mybir
from concourse import bacc
from concourse.tile import TileContext
from concourse.bass_utils import run_bass_kernel_spmd

B, T, E, H = 8, 2048, 128, 4
NF = 1024
NCH = 8
AL = mybir.AluOpType
DT = mybir.dt
AF = mybir.ActivationFunctionType
AX = mybir.AxisListType

_CACHE = {}
LAST_EXEC_NS = None


def _wire_ntff_hook():
    if "antenv.axon_hooks" in sys.modules:
        return
    try:
        mod = types.ModuleType("antenv.axon_hooks")
        _h = [None]
        mod.set_axon_ntff_profile_hook = lambda h: _h.__setitem__(0, h)
        mod.get_axon_ntff_profile_hook = lambda: _h[0]
        sys.modules["antenv.axon_hooks"] = mod
        import antenv
        antenv.axon_hooks = mod
        from trn_agent_boot.trn_boot import _ntff_profile_via_ctypes
        mod.set_axon_ntff_profile_hook(_ntff_profile_via_ctypes("/opt/axon/libaxon_pjrt.so"))
    except Exception:
        pass


def _host_consts():
    i = np.arange(NF, dtype=np.float64)
    ang = np.outer(i, i) * (2.0 * np.pi / T)

    def chunk(a):  # [1024,1024] -> [128, 8*1024] chunk-major
        return np.ascontiguousarray(
            a.reshape(NCH, 128, NF).transpose(1, 0, 2).reshape(128, NCH * NF))

    return {
        "cs": chunk(np.cos(ang).astype(np.float16)),
        "sn": chunk(np.sin(ang).astype(np.float16)),
        "altf": ((-1.0) ** np.arange(NF)).astype(np.float16)[None, :],
        "altp": ((-1.0) ** np.arange(128)).astype(np.float16)[:, None],
        "one": np.ones((1, 1), np.float16),
        "mhalf": np.full((1, 1), -0.5, np.float16),
        "mhrow": np.full((1, NF), -0.5, np.float16),
        "ident": np.eye(128, dtype=np.float32),
    }


def _build():
    nc = bacc.Bacc("TRN2", target_bir_lowering=False, debug=False, num_devices=1)
    f32, f32r, f16, i32, u32 = DT.float32, DT.float32r, DT.float16, DT.int32, DT.uint32

    # all 2D tensors ship host-pre-chunked: [partition, chunk-major free]
    xc_d = nc.dram_tensor("xc", [128, NCH * E], f16, kind="ExternalInput")
    xs_d = nc.dram_tensor("xs", [128, NCH * E], f16, kind="ExternalInput")
    xnyq_d = nc.dram_tensor("xnyq", [1, E], f16, kind="ExternalInput")
    cs_d = nc.dram_tensor("cs", [128, NCH * NF], f16, kind="ExternalInput")
    sn_d = nc.dram_tensor("sn", [128, NCH * NF], f16, kind="ExternalInput")
    altf_d = nc.dram_tensor("altf", [1, NF], f16, kind="ExternalInput")
    altp_d = nc.dram_tensor("altp", [128, 1], f16, kind="ExternalInput")
    one_d = nc.dram_tensor("one", [1, 1], f16, kind="ExternalInput")
    mhalf_d = nc.dram_tensor("mhalf", [1, 1], f16, kind="ExternalInput")
    mhrow_d = nc.dram_tensor("mhrow", [1, NF], f16, kind="ExternalInput")
    id_d = nc.dram_tensor("ident", [128, 128], f32r, kind="ExternalInput")
    wqk_d = nc.dram_tensor("wqk", [128, H * 256], f16, kind="ExternalInput")  # pre-scaled 1/32
    wv_d = nc.dram_tensor("wv", [128, H * E], f16, kind="ExternalInput")
    wo_d = nc.dram_tensor("wo", [128, H * E], f16, kind="ExternalInput")
    bqk_d = nc.dram_tensor("bqk", [1, 2 * H * E], f32, kind="ExternalInput")  # (T/32)*bq | (T/32)*bk
    bv_d = nc.dram_tensor("bv", [E, H], f32, kind="ExternalInput")        # T*bv
    bo_d = nc.dram_tensor("bo", [E, 1], f32, kind="ExternalInput")
    lo_d = nc.dram_tensor("out_lo", [E, NF], f32, kind="ExternalOutput")
    hi_d = nc.dram_tensor("out_hi", [E, NF], f32, kind="ExternalOutput")
    o1024_d = nc.dram_tensor("out_1024", [E, 1], f32, kind="ExternalOutput")

    with TileContext(nc) as tc, ExitStack() as ctx:
        pool = ctx.enter_context(tc.tile_pool(name="main", bufs=1))
        pool2 = ctx.enter_context(tc.tile_pool(name="rot", bufs=2))
        pqk = ctx.enter_context(tc.tile_pool(name="pqk", bufs=2, space="PSUM"))    # [128,1024] tiles
        pb = ctx.enter_context(tc.tile_pool(name="pb", bufs=3, space="PSUM"))      # [128,512] tiles
        prow = ctx.enter_context(tc.tile_pool(name="psrow", bufs=1, space="PSUM"))

        # ---------------- loads ----------------
        xc = pool.tile([128, NCH * E], f16, tag="xc")
        xs = pool.tile([128, NCH * E], f16, tag="xs")
        nc.sync.dma_start(xc[:], xc_d[:])
        nc.sync.dma_start(xs[:], xs_d[:])
        xnyq = pool.tile([1, E], f16, tag="xnyq")
        nc.sync.dma_start(xnyq[:], xnyq_d[:])
        wqk_sb = pool.tile([128, H * 256], f16, tag="wqk")
        nc.sync.dma_start(wqk_sb[:], wqk_d[:])
        altf_sb = pool.tile([1, NF], f16, tag="altf")
        nc.sync.dma_start(altf_sb[:], altf_d[:])
        altp_sb = pool.tile([128, 1], f16, tag="altp")
        nc.sync.dma_start(altp_sb[:], altp_d[:])
        one_sb = pool.tile([1, 1], f16, tag="one")
        nc.sync.dma_start(one_sb[:], one_d[:])

        # big DFT matrices: column-half DMAs so each DFT half starts sooner
        cs_sb = pool.tile([128, NCH * NF], f16, tag="cs")
        sn_sb = pool.tile([128, NCH * NF], f16, tag="sn")
        for half in range(2):
            for t_sb, t_d in ((cs_sb, cs_d), (sn_sb, sn_d)):
                dst = t_sb[:].rearrange("p (a f) -> p a f", a=NCH)[:, :, half * 512:(half + 1) * 512]
                src = t_d[:].rearrange("p (a f) -> p a f", a=NCH)[:, :, half * 512:(half + 1) * 512]
                nc.sync.dma_start(dst, src)

        wv_sb = pool.tile([128, H * E], f16, tag="wv")
        nc.sync.dma_start(wv_sb[:], wv_d[:])
        wo_sb = pool.tile([128, H * E], f16, tag="wo")
        nc.sync.dma_start(wo_sb[:], wo_d[:])
        mhalf_sb = pool.tile([1, 1], f16, tag="mhalf")
        nc.sync.dma_start(mhalf_sb[:], mhalf_d[:])
        mhrow_sb = pool.tile([1, NF], f16, tag="mhrow")
        nc.sync.dma_start(mhrow_sb[:], mhrow_d[:])
        id_sb = pool.tile([128, 128], f32r, tag="ident")
        nc.sync.dma_start(id_sb[:], id_d[:])
        bqk_sb = pool.tile([1, 2 * H * E], f32, tag="bqk")
        nc.sync.dma_start(bqk_sb[:], bqk_d[:])
        bv_sb = pool.tile([128, H], f32, tag="bv")
        nc.sync.dma_start(bv_sb[:], bv_d[:])
        bo_sb = pool.tile([128, 1], f32, tag="bo")
        nc.sync.dma_start(bo_sb[:], bo_d[:])

        # iota/phase prep early (gpsimd idle at start)
        negpi = pool.tile([128, 1], f32, tag="negpi")
        nc.gpsimd.memset(negpi[:], float(-np.pi))
        io_i = pool.tile([128, NF], i32, tag="ioi")
        nc.gpsimd.iota(io_i[:], pattern=[[1, NF]], base=0, channel_multiplier=0)
        io_f = pool.tile([128, NF], f32, tag="iof")
        nc.vector.tensor_copy(io_f[:], io_i[:])

        # ---------------- forward DFT ----------------
        hre = pool.tile([128, NF], f16, tag="hre")
        him = pool.tile([128, NF], f16, tag="him")
        hn = pool.tile([128, 1], f16, tag="hn")
        for s in range(2):
            sl = slice(s * 512, (s + 1) * 512)
            hre_ps = pb.tile([128, 512], f32, tag="b512")
            for a in range(NCH):
                nc.tensor.matmul(hre_ps[:], xc[:, a * E:(a + 1) * E],
                                 cs_sb[:, a * NF + s * 512: a * NF + (s + 1) * 512],
                                 start=(a == 0), stop=False)
            nc.tensor.matmul(hre_ps[:], xnyq[:], altf_sb[:, sl], start=False, stop=True)
            if s == 0:
                nc.scalar.copy(hre[:, sl], hre_ps[:])
            else:
                nc.vector.tensor_copy(hre[:, sl], hre_ps[:])
        for s in range(2):
            sl = slice(s * 512, (s + 1) * 512)
            him_ps = pb.tile([128, 512], f32, tag="b512")
            for a in range(NCH):
                nc.tensor.matmul(him_ps[:], xs[:, a * E:(a + 1) * E],
                                 sn_sb[:, a * NF + s * 512: a * NF + (s + 1) * 512],
                                 start=(a == 0), stop=(a == NCH - 1))
            if s == 0:
                nc.scalar.copy(him[:, sl], him_ps[:])
            else:
                nc.vector.tensor_copy(him[:, sl], him_ps[:])
        hn_ps = prow.tile([128, 1], f32, tag="row")
        for a in range(NCH):
            nc.tensor.matmul(hn_ps[:], xc[:, a * E:(a + 1) * E], altp_sb[:],
                             start=(a == 0), stop=False)
        nc.tensor.matmul(hn_ps[:], xnyq[:], one_sb[:], start=False, stop=True)
        nc.vector.tensor_copy(hn[:], hn_ps[:])

        # ---------------- QK projections + products (fp16, 4x stt) ----------------
        # per (h, comp): two psum tiles of [128,1024] each holding 4 chunks of
        # (q 128 | k 128); evacuated to packed fp16 qr/kr/qi/ki tiles.
        pre_t = pool.tile([128, NCH * E], f16, tag="pre")    # heads 0..2
        pim_t = pool.tile([128, NCH * E], f16, tag="pim")
        pre3 = pool.tile([128, NCH * E], f16, tag="pre3")    # head 3 alone
        pim3 = pool.tile([128, NCH * E], f16, tag="pim3")
        qn_row = pool.tile([1, E], f32, tag="qnrow")
        kn_row = pool.tile([1, E], f32, tag="knrow")
        pn_row = pool.tile([1, E], f32, tag="pnrow")
        pn_f16 = pool.tile([1, E], f16, tag="pnf16")
        pnw = pool.tile([1, E], f32, tag="pnw")
        vn_cols = pool.tile([128, H], f32, tag="vncols")
        vre_t, vim_t = {}, {}

        evac_engs = [nc.scalar, nc.scalar, nc.vector, nc.scalar]
        for h in range(H):
            if True:
                qk = {}
                for c in range(2):
                    hsrc = hre if c == 0 else him
                    for g in range(2):
                        ps = pqk.tile([128, 1024], f32, tag="qk1024")
                        for jj in range(4):
                            j = g * 4 + jj
                            nc.tensor.matmul(ps[:, jj * 256:(jj + 1) * 256],
                                             hsrc[:, j * 128:(j + 1) * 128],
                                             wqk_sb[:, h * 256:(h + 1) * 256],
                                             start=True, stop=True)
                        if c == 0 and g == 0:
                            nc.vector.tensor_add(ps[0:1, 0:128], ps[0:1, 0:128],
                                                 bqk_sb[0:1, h * E:(h + 1) * E])
                            nc.vector.tensor_add(ps[0:1, 128:256], ps[0:1, 128:256],
                                                 bqk_sb[0:1, H * E + h * E:H * E + (h + 1) * E])
                        v3 = ps[:].rearrange("p (jj k e) -> k p jj e", jj=4, k=2)
                        qt = pool2.tile([128, NF], f16, tag=f"q{c}")
                        kt = pool2.tile([128, NF], f16, tag=f"k{c}")
                        if g == 0:
                            qk[c] = (qt, kt)
                        else:
                            qt, kt = qk[c]
                        r2 = lambda ap: ap.rearrange("p (jj e) -> p jj e", jj=4)
                        e0 = evac_engs[(h + 2 * c + g) % 4]
                        e1 = evac_engs[(h + 2 * c + g + 2) % 4]
                        if hasattr(e0, "tensor_copy"):
                            e0.tensor_copy(r2(qt[:, g * 512:(g + 1) * 512]), v3[0])
                        else:
                            e0.copy(r2(qt[:, g * 512:(g + 1) * 512]), v3[0])
                        if hasattr(e1, "tensor_copy"):
                            e1.tensor_copy(r2(kt[:, g * 512:(g + 1) * 512]), v3[1])
                        else:
                            e1.copy(r2(kt[:, g * 512:(g + 1) * 512]), v3[1])
                (qr, kr), (qi, ki) = qk[0], qk[1]
                qr, kr, qi, ki = qr[:], kr[:], qi[:], ki[:]
                t1 = pool2.tile([128, NF], f16, tag="t1")
                t2 = pool2.tile([128, NF], f16, tag="t2")
                t3 = pool2.tile([128, NF], f16, tag="t3")
                t4 = pool2.tile([128, NF], f16, tag="t4")
                nc.vector.tensor_tensor(t1[:], qr, kr, AL.mult)
                nc.vector.tensor_tensor(t2[:], qi, ki, AL.mult)
                nc.vector.tensor_tensor(t3[:], qi, kr, AL.mult)
                nc.vector.tensor_tensor(t4[:], qr, ki, AL.mult)
                if h == 0:
                    nc.vector.tensor_add(pre_t[:], t1[:], t2[:])
                    nc.vector.tensor_sub(pim_t[:], t3[:], t4[:])
                elif h < 3:
                    nc.vector.tensor_add(pre_t[:], pre_t[:], t1[:])
                    nc.vector.tensor_add(pre_t[:], pre_t[:], t2[:])
                    nc.vector.tensor_add(pim_t[:], pim_t[:], t3[:])
                    nc.vector.tensor_sub(pim_t[:], pim_t[:], t4[:])
                else:
                    nc.vector.tensor_add(pre3[:], t1[:], t2[:])
                    nc.vector.tensor_sub(pim3[:], t3[:], t4[:])

                # V spectra for this head (fills PE while vector runs products)
                for c in range(2):
                    hsrc = hre if c == 0 else him
                    vt = pool2.tile([128, NF], f16, tag=f"v{c}{h % 2}")
                    (vre_t if c == 0 else vim_t)[h] = vt
                    for sv in range(2):
                        v_ps = pb.tile([128, 512], f32, tag="b512")
                        nc.tensor.matmul(v_ps[:], wv_sb[:, h * E:(h + 1) * E],
                                         hsrc[:, sv * 512:(sv + 1) * 512], start=True, stop=True)
                        if c == 0 and sv == 0:
                            nc.vector.tensor_add(v_ps[:, 0:1], v_ps[:, 0:1], bv_sb[:, h:h + 1])
                        nc.scalar.copy(vt[:, sv * 512:(sv + 1) * 512], v_ps[:])
                vn_ps = prow.tile([128, 1], f32, tag="row")
                nc.tensor.matmul(vn_ps[:], wv_sb[:, h * E:(h + 1) * E], hn[:], start=True, stop=True)
                nc.scalar.copy(vn_cols[:, h:h + 1], vn_ps[:])

                # Nyquist rows: qn = hn^T @ wq_h ; kn = hn^T @ wk_h (scaled /32 each)
                r_ps = prow.tile([1, 256], f32, tag="row")
                nc.tensor.matmul(r_ps[:], hn[:], wqk_sb[:, h * 256:(h + 1) * 256],
                                 start=True, stop=True)
                nc.scalar.copy(qn_row[:], r_ps[:, 0:128])
                nc.scalar.copy(kn_row[:], r_ps[:, 128:256])
                # unscaled-eo units: pn += 0.5 * qn' * kn'
                if h == 0:
                    nc.vector.scalar_tensor_tensor(pn_row[:], qn_row[:], 0.5, kn_row[:], AL.mult, AL.mult)
                else:
                    nc.vector.scalar_tensor_tensor(pnw[:], qn_row[:], 0.5, kn_row[:], AL.mult, AL.mult)
                    nc.vector.tensor_add(pn_row[:], pn_row[:], pnw[:])
        nc.vector.tensor_copy(pn_f16[:], pn_row[:])

        # ---------------- acm inverse (unscaled by 4x; fixed at softmax) ----------------
        eo_sb = pool.tile([128, T], f32, tag="eo")
        for s in range(2):
            sl = slice(s * 512, (s + 1) * 512)
            e_ps = pb.tile([128, 512], f32, tag="b512")
            o_ps = pb.tile([128, 512], f32, tag="b512")
            # heads 0-2 chunks issue first (run while h3 products finish)
            for j in range(NCH):
                nc.tensor.matmul(e_ps[:], pre_t[:, j * E:(j + 1) * E],
                                 cs_sb[:, j * NF + s * 512: j * NF + (s + 1) * 512],
                                 start=(j == 0), stop=False)
            for j in range(NCH):
                nc.tensor.matmul(o_ps[:], pim_t[:, j * E:(j + 1) * E],
                                 sn_sb[:, j * NF + s * 512: j * NF + (s + 1) * 512],
                                 start=(j == 0), stop=False)
            for j in range(NCH):
                nc.tensor.matmul(e_ps[:], pre3[:, j * E:(j + 1) * E],
                                 cs_sb[:, j * NF + s * 512: j * NF + (s + 1) * 512],
                                 start=False, stop=False)
            nc.tensor.matmul(e_ps[:], pn_f16[:], altf_sb[:, sl], start=False, stop=True)
            for j in range(NCH):
                nc.tensor.matmul(o_ps[:], pim3[:, j * E:(j + 1) * E],
                                 sn_sb[:, j * NF + s * 512: j * NF + (s + 1) * 512],
                                 start=False, stop=(j == NCH - 1))
            e_sb = pool2.tile([128, 512], f32, tag="ecp")
            nc.scalar.copy(e_sb[:], e_ps[:])
            nc.vector.tensor_sub(eo_sb[:, sl], e_sb[:], o_ps[:])
            if s == 0:
                nc.vector.tensor_add(eo_sb[:, NF + 1:NF + 512], e_sb[:, 1:], o_ps[:, 1:])
            else:
                nc.vector.tensor_add(eo_sb[:, NF + 512:2 * NF], e_sb[:], o_ps[:])
        a1024_ps = prow.tile([128, 1], f32, tag="row")
        for j in range(NCH):
            nc.tensor.matmul(a1024_ps[:], pre_t[:, j * E:(j + 1) * E], altp_sb[:],
                             start=(j == 0), stop=False)
        for j in range(NCH):
            nc.tensor.matmul(a1024_ps[:], pre3[:, j * E:(j + 1) * E], altp_sb[:],
                             start=False, stop=False)
        nc.tensor.matmul(a1024_ps[:], pn_f16[:], one_sb[:], start=False, stop=True)
        nc.scalar.copy(eo_sb[:, NF:NF + 1], a1024_ps[:])

        # ---------------- top-k ----------------
        vals = pool.tile([128, 24], f32, tag="vals")
        nc.vector.max(vals[:, 0:8], eo_sb[:])
        nc.vector.match_replace(eo_sb[:], vals[:, 0:8], eo_sb[:], -1e30)
        nc.vector.max(vals[:, 8:16], eo_sb[:])
        nc.vector.match_replace(eo_sb[:], vals[:, 8:16], eo_sb[:], -1e30)
        nc.vector.max(vals[:, 16:24], eo_sb[:])
        idx8 = pool.tile([128, 8], u32, tag="idx8")
        nc.vector.max_index(idx8[:], vals[:, 16:24], eo_sb[:])

        c_i = pool.tile([128, 1], i32, tag="ci")
        nc.vector.tensor_copy(c_i[:], idx8[:, 5:6].bitcast(i32))
        c_neg = pool.tile([128, 1], i32, tag="cneg")
        nc.vector.tensor_scalar(c_neg[:], c_i[:], -1, 3072, AL.mult, AL.add)
        mask = pool.tile([128, 1], i32, tag="mask")
        nc.vector.tensor_scalar(mask[:], c_i[:], 1024, None, AL.is_gt)
        d_i = pool.tile([128, 1], i32, tag="di")
        nc.vector.select(d_i[:], mask[:], c_neg[:], c_i[:])
        d_f = pool.tile([128, 1], f32, tag="df")
        nc.vector.tensor_copy(d_f[:], d_i[:])

        # softmax over 0.25-restored logits
        negv0 = pool.tile([128, 1], f32, tag="negv0")
        nc.vector.tensor_scalar_mul(negv0[:], vals[:, 0:1], -0.25)
        expv = pool.tile([128, 24], f32, tag="expv")
        nc.scalar.activation(expv[:, 0:22], vals[:, 0:22], AF.Exp, bias=negv0[:], scale=0.25)
        den = pool.tile([128, 1], f32, tag="den")
        nc.vector.tensor_reduce(den[:], expv[:, 0:22], AX.X, AL.add)
        rden = pool.tile([128, 1], f32, tag="rden")
        nc.vector.reciprocal(rden[:], den[:])
        wgt = pool.tile([128, 1], f32, tag="wgt")
        nc.vector.tensor_mul(wgt[:], expv[:, 21:22], rden[:])

        # ---------------- phases ----------------
        cw = pool.tile([128, NF], f16, tag="cw")
        sw = pool.tile([128, NF], f16, tag="sw")
        for off, dst in ((0.0, sw), (512.0, cw)):
            mf = pool2.tile([128, NF], f32, tag="ptmp")
            if off == 0.0:
                nc.vector.tensor_scalar(mf[:], io_f[:], d_f[:], None, AL.mult)
            else:
                nc.vector.tensor_scalar(mf[:], io_f[:], d_f[:], off, AL.mult, AL.add)
            mi = pool2.tile([128, NF], i32, tag="ptmp")
            nc.vector.tensor_copy(mi[:], mf[:])
            nc.vector.tensor_scalar(mi[:], mi[:], 2047, None, AL.bitwise_and)
            mf2 = pool2.tile([128, NF], f32, tag="ptmp")
            nc.vector.tensor_copy(mf2[:], mi[:])
            ph = pool2.tile([128, NF], f32, tag="ptmp")
            nc.scalar.activation(ph[:], mf2[:], AF.Sin,
                                 scale=float(np.pi / 1024.0), bias=negpi[:])
            nc.vector.tensor_scalar(dst[:], ph[:], wgt[:], -2.0 / T, AL.mult, AL.mult)
        swn = pool.tile([128, NF], f16, tag="swn")
        nc.vector.tensor_scalar_mul(swn[:], sw[:], -1.0)
        # nyquist scale: (1-2*(d&1)) * wgt / T
        par_i = pool.tile([128, 1], i32, tag="par")
        nc.vector.tensor_scalar(par_i[:], d_i[:], 1, None, AL.bitwise_and)
        parf = pool.tile([128, 1], f32, tag="parf")
        nc.vector.tensor_copy(parf[:], par_i[:])
        nc.vector.tensor_scalar(parf[:], parf[:], -2.0, 1.0, AL.mult, AL.add)
        nys = pool.tile([128, 1], f32, tag="nys")
        nc.vector.tensor_scalar(nys[:], parf[:], wgt[:], 1.0 / T, AL.mult, AL.mult)

        # ---------------- phase multiply + output projection (wo stationary) ----------------
        gn_cols = pool.tile([128, H], f16, tag="gncols")
        og_re = pqk.tile([128, 1024], f32, tag="qk1024")
        og_im = pqk.tile([128, 1024], f32, tag="qk1024")
        for h in range(H):
            vre, vim = vre_t[h], vim_t[h]
            nc.vector.tensor_scalar(gn_cols[:, h:h + 1], vn_cols[:, h:h + 1], nys[:], None, AL.mult)
            m1 = pool2.tile([128, NF], f16, tag="m1")
            m2 = pool2.tile([128, NF], f16, tag="m2")
            m3 = pool2.tile([128, NF], f16, tag="m3")
            m4 = pool2.tile([128, NF], f16, tag="m4")
            nc.vector.tensor_tensor(m1[:], vre[:], cw[:], AL.mult)
            nc.vector.tensor_tensor(m2[:], vim[:], swn[:], AL.mult)
            nc.vector.tensor_tensor(m3[:], vre[:], sw[:], AL.mult)
            nc.vector.tensor_tensor(m4[:], vim[:], cw[:], AL.mult)
            for s in range(2):
                sl = slice(s * 512, (s + 1) * 512)
                nc.tensor.matmul(og_re[:, sl], wo_sb[:, h * E:(h + 1) * E], m1[:, sl],
                                 start=(h == 0), stop=False)
                nc.tensor.matmul(og_re[:, sl], wo_sb[:, h * E:(h + 1) * E], m2[:, sl],
                                 start=False, stop=(h == H - 1))
                nc.tensor.matmul(og_im[:, sl], wo_sb[:, h * E:(h + 1) * E], m3[:, sl],
                                 start=(h == 0), stop=False)
                nc.tensor.matmul(og_im[:, sl], wo_sb[:, h * E:(h + 1) * E], m4[:, sl],
                                 start=False, stop=(h == H - 1))
        ofn_ps = prow.tile([1, E], f32, tag="row")
        for h in range(H):
            nc.tensor.matmul(ofn_ps[:], gn_cols[:, h:h + 1], wo_sb[:, h * E:(h + 1) * E],
                             start=(h == 0), stop=(h == H - 1))
        ofn_row = pool.tile([1, E], f16, tag="ofnrow")
        nc.vector.tensor_copy(ofn_row[:], ofn_ps[:])

        # transpose og [e'', f] -> of [f, e''] via PE (f32r) interleaved with
        # the final-inverse accumulation (e2/o2 span both s-halves in psum)
        g2_re = pool.tile([128, 1024], f32r, tag="g2re")
        g2_im = pool.tile([128, 1024], f32r, tag="g2im")
        nc.scalar.copy(g2_re[:], og_re[:])
        nc.scalar.copy(g2_im[:], og_im[:])
        of_re = pool.tile([128, NCH * E], f16, tag="ofre")
        of_im = pool.tile([128, NCH * E], f16, tag="ofim")
        e2_ps = pqk.tile([128, 1024], f32, tag="qk1024")
        o2_ps = pqk.tile([128, 1024], f32, tag="qk1024")
        id_r = id_sb[:]
        for half in range(4):
            tp = pb.tile([128, 512], f32, tag="b512")
            src = g2_re if half < 2 else g2_im
            dst = of_re if half < 2 else of_im
            base = (half % 2) * 512
            for q in range(4):
                j = (half % 2) * 4 + q
                nc.tensor.transpose(tp[:, q * 128:(q + 1) * 128].bitcast(f32r),
                                    src[:, j * 128:(j + 1) * 128], id_r)
            if half % 2 == 0:
                nc.scalar.copy(dst[:, base:base + 512], tp[:])
            else:
                nc.vector.tensor_copy(dst[:, base:base + 512], tp[:])
            ps_t = e2_ps if half < 2 else o2_ps
            tbl = cs_sb if half < 2 else sn_sb
            for q in range(4):
                j = (half % 2) * 4 + q
                for s in range(2):
                    nc.tensor.matmul(ps_t[:, s * 512:(s + 1) * 512],
                                     dst[:, j * E:(j + 1) * E],
                                     tbl[:, j * NF + s * 512: j * NF + (s + 1) * 512],
                                     start=(half % 2 == 0 and q == 0),
                                     stop=(half == 3 and q == 3))
            if half == 1:
                for s in range(2):
                    sl = slice(s * 512, (s + 1) * 512)
                    nc.tensor.matmul(e2_ps[:, sl], ofn_row[:], altf_sb[:, sl],
                                     start=False, stop=False)
                    nc.tensor.matmul(e2_ps[:, sl], of_re[0:1, 0:E], mhrow_sb[:, sl],
                                     start=False, stop=True)

        for s in range(2):
            sl = slice(s * 512, (s + 1) * 512)
            ep_sb = pool2.tile([128, 512], f32, tag="ecp")
            nc.scalar.copy(ep_sb[:], e2_ps[:, sl])
            out_lo = pool2.tile([128, 512], f32, tag="outlo")
            out_hi = pool2.tile([128, 512], f32, tag="outlo")
            nc.vector.scalar_tensor_tensor(out_lo[:], ep_sb[:], bo_sb[:], o2_ps[:, sl], AL.add, AL.subtract)
            nc.vector.scalar_tensor_tensor(out_hi[:], ep_sb[:], bo_sb[:], o2_ps[:, sl], AL.add, AL.add)
            nc.sync.dma_start(lo_d[:, sl], out_lo[:])
            nc.sync.dma_start(hi_d[:, sl], out_hi[:])
        # t = 1024 row
        o1_ps = prow.tile([128, 1], f32, tag="row")
        for j in range(NCH):
            nc.tensor.matmul(o1_ps[:], of_re[:, j * E:(j + 1) * E], altp_sb[:],
                             start=(j == 0), stop=False)
        nc.tensor.matmul(o1_ps[:], ofn_row[:], one_sb[:], start=False, stop=False)
        nc.tensor.matmul(o1_ps[:], of_re[0:1, 0:E], mhalf_sb[:], start=False, stop=True)
        o1_sb = pool.tile([128, 1], f32, tag="o1sb")
        nc.vector.tensor_scalar(o1_sb[:], o1_ps[:], bo_sb[:], None, AL.add)
        nc.sync.dma_start(o1024_d[:], o1_sb[:])

    nc.compile()
    return nc


def _get_nc():
    if "nc" not in _CACHE:
        _wire_ntff_hook()
        _CACHE["nc"] = _build()
    return _CACHE["nc"]


def kernel(hidden_states, wq, bq, wk, bk, wv, bv, wo, bo):
    global LAST_EXEC_NS
    nc = _get_nc()
    consts = _CACHE.setdefault("consts", _host_consts())

    def chunked(a):
        # [1024, W] -> [128, 8*W] with chunk-major columns (device layout)
        W = a.shape[1]
        return np.ascontiguousarray(
            a.reshape(NCH, 128, W).transpose(1, 0, 2).reshape(128, NCH * W))

    hs = np.ascontiguousarray(hidden_states, dtype=np.float32)
    wqk = np.ascontiguousarray(
        (np.concatenate([wq.transpose(2, 0, 1), wk.transpose(2, 0, 1)], axis=2)
         * (1.0 / 32.0)).transpose(1, 0, 2).reshape(128, H * 256)).astype(np.float16)
    wv_h = np.ascontiguousarray(
        wv.transpose(2, 0, 1).transpose(1, 0, 2).reshape(128, H * E)).astype(np.float16)
    wo_h = np.ascontiguousarray(
        wo.transpose(1, 0, 2).transpose(1, 0, 2).reshape(128, H * E)).astype(np.float16)
    bqk = (np.concatenate([(T * bq.T).reshape(-1), (T * bk.T).reshape(-1)])[None, :]
           * (1.0 / 32.0)).astype(np.float32)                                  # [1, 2*H*E]
    bv_s = np.ascontiguousarray(T * bv, dtype=np.float32)                      # [E, H]
    bo_c = np.ascontiguousarray(bo, dtype=np.float32)[:, None]                 # [E, 1]

    in_maps = []
    for b in range(B):
        x = hs[b]
        xr = np.concatenate([x[0:1], x[:0:-1]])[:NF]
        xc = (x[:NF] + xr)
        xc[0] *= 0.5
        xs = (xr - x[:NF])
        in_maps.append({
            "xc": chunked(xc).astype(np.float16), "xs": chunked(xs).astype(np.float16),
            "xnyq": x[NF:NF + 1].astype(np.float16),
            "cs": consts["cs"], "sn": consts["sn"], "altf": consts["altf"],
            "altp": consts["altp"], "one": consts["one"], "mhalf": consts["mhalf"],
            "mhrow": consts["mhrow"], "ident": consts["ident"],
            "wqk": wqk, "wv": wv_h, "wo": wo_h, "bqk": bqk, "bv": bv_s, "bo": bo_c,
        })

    trace = bool(int(os.environ.get("BASS_KERNEL_TRACE", "0")))
    res = run_bass_kernel_spmd(nc, in_maps, core_ids=list(range(B)), trace=trace)
    LAST_EXEC_NS = res.exec_time_ns
    _CACHE["last_res"] = res

    out = np.empty((B, T, E), dtype=np.float32)
    for b in range(B):
        r = res.results[b]
        out[b, 0:NF] = r["out_lo"].T
        out[b, NF] = r["out_1024"][:, 0]
        out[b, NF + 1:] = r["out_hi"][:, 1:NF][:, ::-1].T
    return out


# revision 39
# speedup vs baseline: 1.1724x; 1.0517x over previous
"""AutoCorrelation (Autoformer-style) Trainium2 Bass kernel, v2.

Sharding: data-parallel over batch — 8 batch elements -> 8 NeuronCores, no
collectives. Each core computes its [2048, 128] output slice independently.

Algorithm (same math as v1, validated vs the reference):
  * Folded real-DFT matrix pair C,S = cos/sin(2*pi*i*j/2048) [1024x1024] in
    fp16 (halves HBM traffic; matmuls run 1 cycle/row with fp32 PSUM).
  * x is folded host-side (xc = x[0:1024]+xr, xs = xr-x, DC row halved);
    Q/K/V obtained by projecting the hidden spectrum; biases enter the DC
    bin; Nyquist bin carried separately.
  * wq/wk are host-scaled by 1/32 so Q*K products fit fp16; the exact 2^-2
    restore lands only on the top-k logits (ordering is scale-invariant).
  * Q/K spectra are evacuated PSUM->SBUF fp16 (split across Scalar/Vector/
    GpSimd) and the complex products + head accumulation run as
    scalar_tensor_tensor ops at 4x DVE rate (fp16, packed, SBUF).
  * top-22 per channel: 3 rounds of max8 + match_replace on fp32; delay
    remapped arithmetically from the permuted acm layout (E-O | E+O).
  * circular roll of V = per-channel phase multiply in frequency domain;
    softmax weight and 2/T fold into the fp16 phase tables.
  * output projection runs with wo stationary (16 LDWEIGHTS, long moving
    streams), then 16 PE transposes restore the [f, e] layout for the
    final inverse DFT.
"""
import os
import sys
import types
from contextlib import ExitStack

sys.path.insert(0, "/opt/trn_rl_repo")

import numpy as np

import concourse.bass as bass
import concourse.mybir as mybir
from concourse import bacc
from concourse.tile import TileContext
from concourse.bass_utils import run_bass_kernel_spmd

B, T, E, H = 8, 2048, 128, 4
NF = 1024
NCH = 8
AL = mybir.AluOpType
DT = mybir.dt
AF = mybir.ActivationFunctionType
AX = mybir.AxisListType

_CACHE = {}
LAST_EXEC_NS = None


def _wire_ntff_hook():
    if "antenv.axon_hooks" in sys.modules:
        return
    try:
        mod = types.ModuleType("antenv.axon_hooks")
        _h = [None]
        mod.set_axon_ntff_profile_hook = lambda h: _h.__setitem__(0, h)
        mod.get_axon_ntff_profile_hook = lambda: _h[0]
        sys.modules["antenv.axon_hooks"] = mod
        import antenv
        antenv.axon_hooks = mod
        from trn_agent_boot.trn_boot import _ntff_profile_via_ctypes
        mod.set_axon_ntff_profile_hook(_ntff_profile_via_ctypes("/opt/axon/libaxon_pjrt.so"))
    except Exception:
        pass


def _host_consts():
    i = np.arange(NF, dtype=np.float64)
    ang = np.outer(i, i) * (2.0 * np.pi / T)

    def chunk(a):  # [1024,1024] -> [128, 8*1024] chunk-major
        return np.ascontiguousarray(
            a.reshape(NCH, 128, NF).transpose(1, 0, 2).reshape(128, NCH * NF))

    return {
        "cs": chunk(np.cos(ang).astype(np.float16)),
        "sn": chunk(np.sin(ang).astype(np.float16)),
        "altf": ((-1.0) ** np.arange(NF)).astype(np.float16)[None, :],
        "altp": ((-1.0) ** np.arange(128)).astype(np.float16)[:, None],
        "one": np.ones((1, 1), np.float16),
        "mhalf": np.full((1, 1), -0.5, np.float16),
        "mhrow": np.full((1, NF), -0.5, np.float16),
        "ident": np.eye(128, dtype=np.float32),
    }


def _build():
    nc = bacc.Bacc("TRN2", target_bir_lowering=False, debug=False, num_devices=1)
    f32, f32r, f16, i32, u32 = DT.float32, DT.float32r, DT.float16, DT.int32, DT.uint32

    # all 2D tensors ship host-pre-chunked: [partition, chunk-major free]
    xc_d = nc.dram_tensor("xc", [128, NCH * E], f16, kind="ExternalInput")
    xs_d = nc.dram_tensor("xs", [128, NCH * E], f16, kind="ExternalInput")
    xnyq_d = nc.dram_tensor("xnyq", [1, E], f16, kind="ExternalInput")
    cs_d = nc.dram_tensor("cs", [128, NCH * NF], f16, kind="ExternalInput")
    sn_d = nc.dram_tensor("sn", [128, NCH * NF], f16, kind="ExternalInput")
    altf_d = nc.dram_tensor("altf", [1, NF], f16, kind="ExternalInput")
    altp_d = nc.dram_tensor("altp", [128, 1], f16, kind="ExternalInput")
    one_d = nc.dram_tensor("one", [1, 1], f16, kind="ExternalInput")
    mhalf_d = nc.dram_tensor("mhalf", [1, 1], f16, kind="ExternalInput")
    mhrow_d = nc.dram_tensor("mhrow", [1, NF], f16, kind="ExternalInput")
    id_d = nc.dram_tensor("ident", [128, 128], f32r, kind="ExternalInput")
    wqk_d = nc.dram_tensor("wqk", [128, H * 256], f16, kind="ExternalInput")  # pre-scaled 1/32
    wv_d = nc.dram_tensor("wv", [128, H * E], f16, kind="ExternalInput")
    wo_d = nc.dram_tensor("wo", [128, H * E], f16, kind="ExternalInput")
    bqk_d = nc.dram_tensor("bqk", [1, 2 * H * E], f32, kind="ExternalInput")  # (T/32)*bq | (T/32)*bk
    bv_d = nc.dram_tensor("bv", [E, H], f32, kind="ExternalInput")        # T*bv
    bo_d = nc.dram_tensor("bo", [E, 1], f32, kind="ExternalInput")
    lo_d = nc.dram_tensor("out_lo", [E, NF], f32, kind="ExternalOutput")
    hi_d = nc.dram_tensor("out_hi", [E, NF], f32, kind="ExternalOutput")
    o1024_d = nc.dram_tensor("out_1024", [E, 1], f32, kind="ExternalOutput")

    with TileContext(nc) as tc, ExitStack() as ctx:
        pool = ctx.enter_context(tc.tile_pool(name="main", bufs=1))
        pool2 = ctx.enter_context(tc.tile_pool(name="rot", bufs=2))
        pqk = ctx.enter_context(tc.tile_pool(name="pqk", bufs=2, space="PSUM"))    # [128,1024] tiles
        pb = ctx.enter_context(tc.tile_pool(name="pb", bufs=3, space="PSUM"))      # [128,512] tiles
        prow = ctx.enter_context(tc.tile_pool(name="psrow", bufs=1, space="PSUM"))

        # ---------------- loads ----------------
        xc = pool.tile([128, NCH * E], f16, tag="xc")
        xs = pool.tile([128, NCH * E], f16, tag="xs")
        nc.sync.dma_start(xc[:], xc_d[:])
        nc.sync.dma_start(xs[:], xs_d[:])
        xnyq = pool.tile([1, E], f16, tag="xnyq")
        nc.sync.dma_start(xnyq[:], xnyq_d[:])
        wqk_sb = pool.tile([128, H * 256], f16, tag="wqk")
        nc.sync.dma_start(wqk_sb[:], wqk_d[:])
        altf_sb = pool.tile([1, NF], f16, tag="altf")
        nc.sync.dma_start(altf_sb[:], altf_d[:])
        altp_sb = pool.tile([128, 1], f16, tag="altp")
        nc.sync.dma_start(altp_sb[:], altp_d[:])
        one_sb = pool.tile([1, 1], f16, tag="one")
        nc.sync.dma_start(one_sb[:], one_d[:])

        # big DFT matrices: column-half DMAs so each DFT half starts sooner
        cs_sb = pool.tile([128, NCH * NF], f16, tag="cs")
        sn_sb = pool.tile([128, NCH * NF], f16, tag="sn")
        for half in range(2):
            for t_sb, t_d in ((cs_sb, cs_d), (sn_sb, sn_d)):
                dst = t_sb[:].rearrange("p (a f) -> p a f", a=NCH)[:, :, half * 512:(half + 1) * 512]
                src = t_d[:].rearrange("p (a f) -> p a f", a=NCH)[:, :, half * 512:(half + 1) * 512]
                nc.sync.dma_start(dst, src)

        wv_sb = pool.tile([128, H * E], f16, tag="wv")
        nc.sync.dma_start(wv_sb[:], wv_d[:])
        wo_sb = pool.tile([128, H * E], f16, tag="wo")
        nc.sync.dma_start(wo_sb[:], wo_d[:])
        mhalf_sb = pool.tile([1, 1], f16, tag="mhalf")
        nc.sync.dma_start(mhalf_sb[:], mhalf_d[:])
        mhrow_sb = pool.tile([1, NF], f16, tag="mhrow")
        nc.sync.dma_start(mhrow_sb[:], mhrow_d[:])
        id_sb = pool.tile([128, 128], f32r, tag="ident")
        nc.sync.dma_start(id_sb[:], id_d[:])
        bqk_sb = pool.tile([1, 2 * H * E], f32, tag="bqk")
        nc.sync.dma_start(bqk_sb[:], bqk_d[:])
        bv_sb = pool.tile([128, H], f32, tag="bv")
        nc.sync.dma_start(bv_sb[:], bv_d[:])
        bo_sb = pool.tile([128, 1], f32, tag="bo")
        nc.sync.dma_start(bo_sb[:], bo_d[:])

        # iota/phase prep early (gpsimd idle at start)
        negpi = pool.tile([128, 1], f32, tag="negpi")
        nc.gpsimd.memset(negpi[:], float(-np.pi))
        io_i = pool.tile([128, NF], i32, tag="ioi")
        nc.gpsimd.iota(io_i[:], pattern=[[1, NF]], base=0, channel_multiplier=0)
        io_f = pool.tile([128, NF], f32, tag="iof")
        nc.vector.tensor_copy(io_f[:], io_i[:])

        # ---------------- forward DFT ----------------
        hre = pool.tile([128, NF], f16, tag="hre")
        him = pool.tile([128, NF], f16, tag="him")
        hn = pool.tile([128, 1], f16, tag="hn")
        for s in range(2):
            sl = slice(s * 512, (s + 1) * 512)
            hre_ps = pb.tile([128, 512], f32, tag="b512")
            for a in range(NCH):
                nc.tensor.matmul(hre_ps[:], xc[:, a * E:(a + 1) * E],
                                 cs_sb[:, a * NF + s * 512: a * NF + (s + 1) * 512],
                                 start=(a == 0), stop=False)
            nc.tensor.matmul(hre_ps[:], xnyq[:], altf_sb[:, sl], start=False, stop=True)
            if s == 0:
                nc.scalar.copy(hre[:, sl], hre_ps[:])
            else:
                nc.vector.tensor_copy(hre[:, sl], hre_ps[:])
        for s in range(2):
            sl = slice(s * 512, (s + 1) * 512)
            him_ps = pb.tile([128, 512], f32, tag="b512")
            for a in range(NCH):
                nc.tensor.matmul(him_ps[:], xs[:, a * E:(a + 1) * E],
                                 sn_sb[:, a * NF + s * 512: a * NF + (s + 1) * 512],
                                 start=(a == 0), stop=(a == NCH - 1))
            if s == 0:
                nc.scalar.copy(him[:, sl], him_ps[:])
            else:
                nc.vector.tensor_copy(him[:, sl], him_ps[:])
        hn_ps = prow.tile([128, 1], f32, tag="row")
        for a in range(NCH):
            nc.tensor.matmul(hn_ps[:], xc[:, a * E:(a + 1) * E], altp_sb[:],
                             start=(a == 0), stop=False)
        nc.tensor.matmul(hn_ps[:], xnyq[:], one_sb[:], start=False, stop=True)
        nc.vector.tensor_copy(hn[:], hn_ps[:])

        # ---------------- QK projections + products (fp16, 4x stt) ----------------
        # per (h, comp): two psum tiles of [128,1024] each holding 4 chunks of
        # (q 128 | k 128); evacuated to packed fp16 qr/kr/qi/ki tiles.
        pre_t = pool.tile([128, NCH * E], f16, tag="pre")    # heads 0..2
        pim_t = pool.tile([128, NCH * E], f16, tag="pim")
        pre3 = pool.tile([128, NCH * E], f16, tag="pre3")    # head 3 alone
        pim3 = pool.tile([128, NCH * E], f16, tag="pim3")
        qn_row = pool.tile([1, E], f32, tag="qnrow")
        kn_row = pool.tile([1, E], f32, tag="knrow")
        pn_row = pool.tile([1, E], f32, tag="pnrow")
        pn_f16 = pool.tile([1, E], f16, tag="pnf16")
        pnw = pool.tile([1, E], f32, tag="pnw")
        vn_cols = pool.tile([128, H], f32, tag="vncols")
        vre_t, vim_t = {}, {}

        evac_engs = [nc.scalar, nc.scalar, nc.vector, nc.scalar]
        for h in range(H):
            if True:
                qk = {}
                for c in range(2):
                    hsrc = hre if c == 0 else him
                    for g in range(2):
                        ps = pqk.tile([128, 1024], f32, tag="qk1024")
                        for jj in range(4):
                            j = g * 4 + jj
                            nc.tensor.matmul(ps[:, jj * 256:(jj + 1) * 256],
                                             hsrc[:, j * 128:(j + 1) * 128],
                                             wqk_sb[:, h * 256:(h + 1) * 256],
                                             start=True, stop=True)
                        if c == 0 and g == 0:
                            nc.vector.tensor_add(ps[0:1, 0:128], ps[0:1, 0:128],
                                                 bqk_sb[0:1, h * E:(h + 1) * E])
                            nc.vector.tensor_add(ps[0:1, 128:256], ps[0:1, 128:256],
                                                 bqk_sb[0:1, H * E + h * E:H * E + (h + 1) * E])
                        v3 = ps[:].rearrange("p (jj k e) -> k p jj e", jj=4, k=2)
                        qt = pool2.tile([128, NF], f16, tag=f"q{c}")
                        kt = pool2.tile([128, NF], f16, tag=f"k{c}")
                        if g == 0:
                            qk[c] = (qt, kt)
                        else:
                            qt, kt = qk[c]
                        r2 = lambda ap: ap.rearrange("p (jj e) -> p jj e", jj=4)
                        e0 = evac_engs[(h + 2 * c + g) % 4]
                        e1 = evac_engs[(h + 2 * c + g + 2) % 4]
                        if hasattr(e0, "tensor_copy"):
                            e0.tensor_copy(r2(qt[:, g * 512:(g + 1) * 512]), v3[0])
                        else:
                            e0.copy(r2(qt[:, g * 512:(g + 1) * 512]), v3[0])
                        if hasattr(e1, "tensor_copy"):
                            e1.tensor_copy(r2(kt[:, g * 512:(g + 1) * 512]), v3[1])
                        else:
                            e1.copy(r2(kt[:, g * 512:(g + 1) * 512]), v3[1])
                (qr, kr), (qi, ki) = qk[0], qk[1]
                qr, kr, qi, ki = qr[:], kr[:], qi[:], ki[:]
                t1 = pool2.tile([128, NF], f16, tag="t1")
                t2 = pool2.tile([128, NF], f16, tag="t2")
                t3 = pool2.tile([128, NF], f16, tag="t3")
                t4 = pool2.tile([128, NF], f16, tag="t4")
                nc.vector.tensor_tensor(t1[:], qr, kr, AL.mult)
                nc.vector.tensor_tensor(t2[:], qi, ki, AL.mult)
                nc.vector.tensor_tensor(t3[:], qi, kr, AL.mult)
                nc.vector.tensor_tensor(t4[:], qr, ki, AL.mult)
                if h == 0:
                    nc.vector.tensor_add(pre_t[:], t1[:], t2[:])
                    nc.vector.tensor_sub(pim_t[:], t3[:], t4[:])
                elif h < 3:
                    nc.vector.tensor_add(pre_t[:], pre_t[:], t1[:])
                    nc.vector.tensor_add(pre_t[:], pre_t[:], t2[:])
                    nc.vector.tensor_add(pim_t[:], pim_t[:], t3[:])
                    nc.vector.tensor_sub(pim_t[:], pim_t[:], t4[:])
                else:
                    nc.vector.tensor_add(pre3[:], t1[:], t2[:])
                    nc.vector.tensor_sub(pim3[:], t3[:], t4[:])

                # V spectra for this head (fills PE while vector runs products)
                for c in range(2):
                    hsrc = hre if c == 0 else him
                    vt = pool2.tile([128, NF], f16, tag=f"v{c}{h % 2}")
                    (vre_t if c == 0 else vim_t)[h] = vt
                    for sv in range(2):
                        v_ps = pb.tile([128, 512], f32, tag="b512")
                        nc.tensor.matmul(v_ps[:], wv_sb[:, h * E:(h + 1) * E],
                                         hsrc[:, sv * 512:(sv + 1) * 512], start=True, stop=True)
                        if c == 0 and sv == 0:
                            nc.vector.tensor_add(v_ps[:, 0:1], v_ps[:, 0:1], bv_sb[:, h:h + 1])
                        nc.scalar.copy(vt[:, sv * 512:(sv + 1) * 512], v_ps[:])
                vn_ps = prow.tile([128, 1], f32, tag="row")
                nc.tensor.matmul(vn_ps[:], wv_sb[:, h * E:(h + 1) * E], hn[:], start=True, stop=True)
                nc.scalar.copy(vn_cols[:, h:h + 1], vn_ps[:])

                # Nyquist rows: qn = hn^T @ wq_h ; kn = hn^T @ wk_h (scaled /32 each)
                r_ps = prow.tile([1, 256], f32, tag="row")
                nc.tensor.matmul(r_ps[:], hn[:], wqk_sb[:, h * 256:(h + 1) * 256],
                                 start=True, stop=True)
                nc.scalar.copy(qn_row[:], r_ps[:, 0:128])
                nc.scalar.copy(kn_row[:], r_ps[:, 128:256])
                # unscaled-eo units: pn += 0.5 * qn' * kn'
                if h == 0:
                    nc.vector.scalar_tensor_tensor(pn_row[:], qn_row[:], 0.5, kn_row[:], AL.mult, AL.mult)
                else:
                    nc.vector.scalar_tensor_tensor(pnw[:], qn_row[:], 0.5, kn_row[:], AL.mult, AL.mult)
                    nc.vector.tensor_add(pn_row[:], pn_row[:], pnw[:])
        nc.vector.tensor_copy(pn_f16[:], pn_row[:])

        # ---------------- acm inverse (unscaled by 4x; fixed at softmax) ----------------
        eo_sb = pool.tile([128, T], f32, tag="eo")
        e_acm = pqk.tile([128, 1024], f32, tag="qk1024")
        o_acm = pqk.tile([128, 1024], f32, tag="qk1024")
        # heads 0-2 chunks issue first (run while h3 products finish)
        for s in range(2):
            sl = slice(s * 512, (s + 1) * 512)
            for j in range(NCH):
                nc.tensor.matmul(e_acm[:, sl], pre_t[:, j * E:(j + 1) * E],
                                 cs_sb[:, j * NF + s * 512: j * NF + (s + 1) * 512],
                                 start=(j == 0), stop=False)
            for j in range(NCH):
                nc.tensor.matmul(o_acm[:, sl], pim_t[:, j * E:(j + 1) * E],
                                 sn_sb[:, j * NF + s * 512: j * NF + (s + 1) * 512],
                                 start=(j == 0), stop=False)
        # acm[t=1024]: small serial chain, kept off the top-k critical path
        a1024_ps = prow.tile([128, 1], f32, tag="row")
        for j in range(NCH):
            nc.tensor.matmul(a1024_ps[:], pre_t[:, j * E:(j + 1) * E], altp_sb[:],
                             start=(j == 0), stop=False)
        for j in range(NCH):
            nc.tensor.matmul(a1024_ps[:], pre3[:, j * E:(j + 1) * E], altp_sb[:],
                             start=False, stop=False)
        nc.tensor.matmul(a1024_ps[:], pn_f16[:], one_sb[:], start=False, stop=True)
        nc.scalar.copy(eo_sb[:, NF:NF + 1], a1024_ps[:])
        # head-3 completion + combine per s-half
        for s in range(2):
            sl = slice(s * 512, (s + 1) * 512)
            for j in range(NCH):
                nc.tensor.matmul(e_acm[:, sl], pre3[:, j * E:(j + 1) * E],
                                 cs_sb[:, j * NF + s * 512: j * NF + (s + 1) * 512],
                                 start=False, stop=False)
            nc.tensor.matmul(e_acm[:, sl], pn_f16[:], altf_sb[:, sl], start=False, stop=True)
            for j in range(NCH):
                nc.tensor.matmul(o_acm[:, sl], pim3[:, j * E:(j + 1) * E],
                                 sn_sb[:, j * NF + s * 512: j * NF + (s + 1) * 512],
                                 start=False, stop=(j == NCH - 1))
            e_sb = pool2.tile([128, 512], f32, tag="ecp")
            nc.scalar.copy(e_sb[:], e_acm[:, sl])
            nc.vector.tensor_sub(eo_sb[:, sl], e_sb[:], o_acm[:, sl])
            if s == 0:
                nc.vector.tensor_add(eo_sb[:, NF + 1:NF + 512], e_sb[:, 1:], o_acm[:, sl][:, 1:])
            else:
                nc.vector.tensor_add(eo_sb[:, NF + 512:2 * NF], e_sb[:], o_acm[:, sl])

        # ---------------- top-k ----------------
        vals = pool.tile([128, 24], f32, tag="vals")
        nc.vector.max(vals[:, 0:8], eo_sb[:])
        nc.vector.match_replace(eo_sb[:], vals[:, 0:8], eo_sb[:], -1e30)
        nc.vector.max(vals[:, 8:16], eo_sb[:])
        nc.vector.match_replace(eo_sb[:], vals[:, 8:16], eo_sb[:], -1e30)
        nc.vector.max(vals[:, 16:24], eo_sb[:])
        idx8 = pool.tile([128, 8], u32, tag="idx8")
        nc.vector.max_index(idx8[:], vals[:, 16:24], eo_sb[:])

        c_i = pool.tile([128, 1], i32, tag="ci")
        nc.vector.tensor_copy(c_i[:], idx8[:, 5:6].bitcast(i32))
        c_neg = pool.tile([128, 1], i32, tag="cneg")
        nc.vector.tensor_scalar(c_neg[:], c_i[:], -1, 3072, AL.mult, AL.add)
        mask = pool.tile([128, 1], i32, tag="mask")
        nc.vector.tensor_scalar(mask[:], c_i[:], 1024, None, AL.is_gt)
        d_i = pool.tile([128, 1], i32, tag="di")
        nc.vector.select(d_i[:], mask[:], c_neg[:], c_i[:])
        d_f = pool.tile([128, 1], f32, tag="df")
        nc.vector.tensor_copy(d_f[:], d_i[:])

        # softmax over 0.25-restored logits
        negv0 = pool.tile([128, 1], f32, tag="negv0")
        nc.vector.tensor_scalar_mul(negv0[:], vals[:, 0:1], -0.25)
        expv = pool.tile([128, 24], f32, tag="expv")
        nc.scalar.activation(expv[:, 0:22], vals[:, 0:22], AF.Exp, bias=negv0[:], scale=0.25)
        den = pool.tile([128, 1], f32, tag="den")
        nc.vector.tensor_reduce(den[:], expv[:, 0:22], AX.X, AL.add)
        rden = pool.tile([128, 1], f32, tag="rden")
        nc.vector.reciprocal(rden[:], den[:])
        wgt = pool.tile([128, 1], f32, tag="wgt")
        nc.vector.tensor_mul(wgt[:], expv[:, 21:22], rden[:])

        # ---------------- phases ----------------
        cw = pool.tile([128, NF], f16, tag="cw")
        sw = pool.tile([128, NF], f16, tag="sw")
        for off, dst in ((0.0, sw), (512.0, cw)):
            mf = pool2.tile([128, NF], f32, tag="ptmp")
            if off == 0.0:
                nc.vector.tensor_scalar(mf[:], io_f[:], d_f[:], None, AL.mult)
            else:
                nc.vector.tensor_scalar(mf[:], io_f[:], d_f[:], off, AL.mult, AL.add)
            mi = pool2.tile([128, NF], i32, tag="ptmp")
            nc.vector.tensor_copy(mi[:], mf[:])
            nc.vector.tensor_scalar(mi[:], mi[:], 2047, None, AL.bitwise_and)
            mf2 = pool2.tile([128, NF], f32, tag="ptmp")
            nc.vector.tensor_copy(mf2[:], mi[:])
            ph = pool2.tile([128, NF], f32, tag="ptmp")
            nc.scalar.activation(ph[:], mf2[:], AF.Sin,
                                 scale=float(np.pi / 1024.0), bias=negpi[:])
            nc.vector.tensor_scalar(dst[:], ph[:], wgt[:], -2.0 / T, AL.mult, AL.mult)
        swn = pool.tile([128, NF], f16, tag="swn")
        nc.vector.tensor_scalar_mul(swn[:], sw[:], -1.0)
        # nyquist scale: (1-2*(d&1)) * wgt / T
        par_i = pool.tile([128, 1], i32, tag="par")
        nc.vector.tensor_scalar(par_i[:], d_i[:], 1, None, AL.bitwise_and)
        parf = pool.tile([128, 1], f32, tag="parf")
        nc.vector.tensor_copy(parf[:], par_i[:])
        nc.vector.tensor_scalar(parf[:], parf[:], -2.0, 1.0, AL.mult, AL.add)
        nys = pool.tile([128, 1], f32, tag="nys")
        nc.vector.tensor_scalar(nys[:], parf[:], wgt[:], 1.0 / T, AL.mult, AL.mult)

        # ---------------- phase multiply + output projection (wo stationary) ----------------
        gn_cols = pool.tile([128, H], f16, tag="gncols")
        og_re = pqk.tile([128, 1024], f32, tag="qk1024")
        og_im = pqk.tile([128, 1024], f32, tag="qk1024")
        for h in range(H):
            vre, vim = vre_t[h], vim_t[h]
            nc.vector.tensor_scalar(gn_cols[:, h:h + 1], vn_cols[:, h:h + 1], nys[:], None, AL.mult)
            m1 = pool2.tile([128, NF], f16, tag="m1")
            m2 = pool2.tile([128, NF], f16, tag="m2")
            m3 = pool2.tile([128, NF], f16, tag="m3")
            m4 = pool2.tile([128, NF], f16, tag="m4")
            nc.vector.tensor_tensor(m1[:], vre[:], cw[:], AL.mult)
            nc.vector.tensor_tensor(m2[:], vim[:], swn[:], AL.mult)
            nc.vector.tensor_tensor(m3[:], vre[:], sw[:], AL.mult)
            nc.vector.tensor_tensor(m4[:], vim[:], cw[:], AL.mult)
            for s in range(2):
                sl = slice(s * 512, (s + 1) * 512)
                nc.tensor.matmul(og_re[:, sl], wo_sb[:, h * E:(h + 1) * E], m1[:, sl],
                                 start=(h == 0), stop=False)
                nc.tensor.matmul(og_re[:, sl], wo_sb[:, h * E:(h + 1) * E], m2[:, sl],
                                 start=False, stop=(h == H - 1))
                nc.tensor.matmul(og_im[:, sl], wo_sb[:, h * E:(h + 1) * E], m3[:, sl],
                                 start=(h == 0), stop=False)
                nc.tensor.matmul(og_im[:, sl], wo_sb[:, h * E:(h + 1) * E], m4[:, sl],
                                 start=False, stop=(h == H - 1))
        ofn_ps = prow.tile([1, E], f32, tag="row")
        for h in range(H):
            nc.tensor.matmul(ofn_ps[:], gn_cols[:, h:h + 1], wo_sb[:, h * E:(h + 1) * E],
                             start=(h == 0), stop=(h == H - 1))
        ofn_row = pool.tile([1, E], f16, tag="ofnrow")
        nc.vector.tensor_copy(ofn_row[:], ofn_ps[:])

        # transpose og [e'', f] -> of [f, e''] via PE (f32r) interleaved with
        # the final-inverse accumulation (e2/o2 span both s-halves in psum)
        g2_re = pool.tile([128, 1024], f32r, tag="g2re")
        g2_im = pool.tile([128, 1024], f32r, tag="g2im")
        nc.scalar.copy(g2_re[:], og_re[:])
        nc.scalar.copy(g2_im[:], og_im[:])
        of_re = pool.tile([128, NCH * E], f16, tag="ofre")
        of_im = pool.tile([128, NCH * E], f16, tag="ofim")
        e2_ps = pqk.tile([128, 1024], f32, tag="qk1024")
        o2_ps = pqk.tile([128, 1024], f32, tag="qk1024")
        id_r = id_sb[:]
        for half in range(4):
            tp = pb.tile([128, 512], f32, tag="b512")
            src = g2_re if half < 2 else g2_im
            dst = of_re if half < 2 else of_im
            base = (half % 2) * 512
            for q in range(4):
                j = (half % 2) * 4 + q
                nc.tensor.transpose(tp[:, q * 128:(q + 1) * 128].bitcast(f32r),
                                    src[:, j * 128:(j + 1) * 128], id_r)
            if half % 2 == 0:
                nc.scalar.copy(dst[:, base:base + 512], tp[:])
            else:
                nc.vector.tensor_copy(dst[:, base:base + 512], tp[:])
            ps_t = e2_ps if half < 2 else o2_ps
            tbl = cs_sb if half < 2 else sn_sb
            for q in range(4):
                j = (half % 2) * 4 + q
                for s in range(2):
                    nc.tensor.matmul(ps_t[:, s * 512:(s + 1) * 512],
                                     dst[:, j * E:(j + 1) * E],
                                     tbl[:, j * NF + s * 512: j * NF + (s + 1) * 512],
                                     start=(half % 2 == 0 and q == 0),
                                     stop=(half == 3 and q == 3))
            if half == 1:
                for s in range(2):
                    sl = slice(s * 512, (s + 1) * 512)
                    nc.tensor.matmul(e2_ps[:, sl], ofn_row[:], altf_sb[:, sl],
                                     start=False, stop=False)
                    nc.tensor.matmul(e2_ps[:, sl], of_re[0:1, 0:E], mhrow_sb[:, sl],
                                     start=False, stop=True)

        for s in range(2):
            sl = slice(s * 512, (s + 1) * 512)
            ep_sb = pool2.tile([128, 512], f32, tag="ecp")
            nc.scalar.copy(ep_sb[:], e2_ps[:, sl])
            out_lo = pool2.tile([128, 512], f32, tag="outlo")
            out_hi = pool2.tile([128, 512], f32, tag="outlo")
            nc.vector.scalar_tensor_tensor(out_lo[:], ep_sb[:], bo_sb[:], o2_ps[:, sl], AL.add, AL.subtract)
            nc.vector.scalar_tensor_tensor(out_hi[:], ep_sb[:], bo_sb[:], o2_ps[:, sl], AL.add, AL.add)
            nc.sync.dma_start(lo_d[:, sl], out_lo[:])
            nc.sync.dma_start(hi_d[:, sl], out_hi[:])
        # t = 1024 row
        o1_ps = prow.tile([128, 1], f32, tag="row")
        for j in range(NCH):
            nc.tensor.matmul(o1_ps[:], of_re[:, j * E:(j + 1) * E], altp_sb[:],
                             start=(j == 0), stop=False)
        nc.tensor.matmul(o1_ps[:], ofn_row[:], one_sb[:], start=False, stop=False)
        nc.tensor.matmul(o1_ps[:], of_re[0:1, 0:E], mhalf_sb[:], start=False, stop=True)
        o1_sb = pool.tile([128, 1], f32, tag="o1sb")
        nc.vector.tensor_scalar(o1_sb[:], o1_ps[:], bo_sb[:], None, AL.add)
        nc.sync.dma_start(o1024_d[:], o1_sb[:])

    nc.compile()
    return nc


def _get_nc():
    if "nc" not in _CACHE:
        _wire_ntff_hook()
        _CACHE["nc"] = _build()
    return _CACHE["nc"]


def kernel(hidden_states, wq, bq, wk, bk, wv, bv, wo, bo):
    global LAST_EXEC_NS
    nc = _get_nc()
    consts = _CACHE.setdefault("consts", _host_consts())

    def chunked(a):
        # [1024, W] -> [128, 8*W] with chunk-major columns (device layout)
        W = a.shape[1]
        return np.ascontiguousarray(
            a.reshape(NCH, 128, W).transpose(1, 0, 2).reshape(128, NCH * W))

    hs = np.ascontiguousarray(hidden_states, dtype=np.float32)
    wqk = np.ascontiguousarray(
        (np.concatenate([wq.transpose(2, 0, 1), wk.transpose(2, 0, 1)], axis=2)
         * (1.0 / 32.0)).transpose(1, 0, 2).reshape(128, H * 256)).astype(np.float16)
    wv_h = np.ascontiguousarray(
        wv.transpose(2, 0, 1).transpose(1, 0, 2).reshape(128, H * E)).astype(np.float16)
    wo_h = np.ascontiguousarray(
        wo.transpose(1, 0, 2).transpose(1, 0, 2).reshape(128, H * E)).astype(np.float16)
    bqk = (np.concatenate([(T * bq.T).reshape(-1), (T * bk.T).reshape(-1)])[None, :]
           * (1.0 / 32.0)).astype(np.float32)                                  # [1, 2*H*E]
    bv_s = np.ascontiguousarray(T * bv, dtype=np.float32)                      # [E, H]
    bo_c = np.ascontiguousarray(bo, dtype=np.float32)[:, None]                 # [E, 1]

    in_maps = []
    for b in range(B):
        x = hs[b]
        xr = np.concatenate([x[0:1], x[:0:-1]])[:NF]
        xc = (x[:NF] + xr)
        xc[0] *= 0.5
        xs = (xr - x[:NF])
        in_maps.append({
            "xc": chunked(xc).astype(np.float16), "xs": chunked(xs).astype(np.float16),
            "xnyq": x[NF:NF + 1].astype(np.float16),
            "cs": consts["cs"], "sn": consts["sn"], "altf": consts["altf"],
            "altp": consts["altp"], "one": consts["one"], "mhalf": consts["mhalf"],
            "mhrow": consts["mhrow"], "ident": consts["ident"],
            "wqk": wqk, "wv": wv_h, "wo": wo_h, "bqk": bqk, "bv": bv_s, "bo": bo_c,
        })

    trace = bool(int(os.environ.get("BASS_KERNEL_TRACE", "0")))
    res = run_bass_kernel_spmd(nc, in_maps, core_ids=list(range(B)), trace=trace)
    LAST_EXEC_NS = res.exec_time_ns
    _CACHE["last_res"] = res

    out = np.empty((B, T, E), dtype=np.float32)
    for b in range(B):
        r = res.results[b]
        out[b, 0:NF] = r["out_lo"].T
        out[b, NF] = r["out_1024"][:, 0]
        out[b, NF + 1:] = r["out_hi"][:, 1:NF][:, ::-1].T
    return out
